# revision 1
# baseline (speedup 1.0000x reference)
"""BiLSTM-CRF forward loss on 8 TRN2 NeuronCores (Bass/Tile).

Sharding: data-parallel over batch (32 seqs -> 4 per core), params replicated.
Each core computes per-sequence CRF numerator, logZ and aux-CE partials; the
host combines them into the scalar loss (pure unsharding arithmetic).
"""
import sys

import numpy as np

try:
    import concourse  # noqa: F401
except ImportError:  # pragma: no cover
    sys.path.insert(0, "/opt/trn_rl_repo")

import ml_dtypes
from contextlib import ExitStack

import concourse.bass as bass
import concourse.bacc as bacc
import concourse.mybir as mybir
import concourse.tile as tile
from concourse.bass_utils import run_bass_kernel_spmd

F32 = mybir.dt.float32
BF16 = mybir.dt.bfloat16
I32 = mybir.dt.int32
AF = mybir.ActivationFunctionType
ALU = mybir.AluOpType
AX = mybir.AxisListType

B, S, E, H, T, V = 32, 256, 256, 512, 17, 50000
NC = 8
BL = B // NC          # 4 local sequences per core
TOK = BL * S          # 1024 local tokens, flat index = 256*b + t
G = 4 * H             # 2048 gate rows
GC = G // 128         # 16 gate chunks
KH = H // 128         # 4 hidden chunks
RENORM = 8            # CRF renorm period
NREN = (S - 1) // RENORM  # 31 renorm events (t = 8,16,...,248)

_CACHE = {}


def _build_nc(steps=S):
    nc = bacc.Bacc(None, target_bir_lowering=False, num_devices=NC)
    d = {}
    P = nc.declare_dram_parameter
    d["x_idx"] = P("x_idx", [128, TOK // 128], I32, isOutput=False)
    d["emb"] = P("emb", [V, E], F32, isOutput=False)
    d["wih0T"] = P("wih0T", [2, 2, 128, G], BF16, isOutput=False)
    d["whh0T"] = P("whh0T", [2, 4, 128, G], BF16, isOutput=False)
    d["wih1T"] = P("wih1T", [2, 8, 128, G], BF16, isOutput=False)
    d["whh1T"] = P("whh1T", [2, 4, 128, G], BF16, isOutput=False)
    d["hwT"] = P("hwT", [2, 8, 128, 2 * H], BF16, isOutput=False)  # [t/h, k, p, o]
    d["fcwT"] = P("fcwT", [128, 8 * T], BF16, isOutput=False)  # [p, k*T]
    d["aux128"] = P("aux128", [128, 80], F32, isOutput=False)
    d["aux17"] = P("aux17", [T, 1120], F32, isOutput=False)
    d["aux1"] = P("aux1", [1, NREN * BL + TOK], F32, isOutput=False)
    d["msel"] = P("msel", [T, TOK], mybir.dt.uint8, isOutput=False)
    out_d = P("out", [4, BL], F32, isOutput=True)

    with tile.TileContext(nc) as tc, ExitStack() as ctx:
        pp = ctx.enter_context(tc.tile_pool(name="persist", bufs=1))
        wp = ctx.enter_context(tc.tile_pool(name="wts", bufs=1))
        sp = ctx.enter_context(tc.tile_pool(name="small", bufs=2))
        op = ctx.enter_context(tc.tile_pool(name="once", bufs=1))
        ps = ctx.enter_context(tc.tile_pool(name="psum", bufs=2, space="PSUM"))

        dma = nc.sync.dma_start

        # ---- static small loads -------------------------------------------------
        x_sb = pp.tile([128, TOK // 128], I32, tag="xidx")
        dma(x_sb[:], d["x_idx"][:])
        fcw_sb = pp.tile([128, 8, T], BF16, tag="fcw")
        dma(fcw_sb[:], d["fcwT"][:].rearrange("p (k t) -> p k t", k=8))
        aux128_sb = pp.tile([128, 80], F32, tag="aux128")
        dma(aux128_sb[:], d["aux128"][:])
        aux17_sb = pp.tile([T, 1120], F32, tag="aux17")
        dma(aux17_sb[:], d["aux17"][:])
        aux1_sb = pp.tile([1, NREN * BL + TOK], F32, tag="aux1")
        dma(aux1_sb[:], d["aux1"][:])
        msel_sb = pp.tile([T, BL, S], mybir.dt.uint8, tag="msel")
        dma(msel_sb[:], d["msel"][:].rearrange("t (b s) -> t b s", b=BL))

        def b0v(dd, c):
            return aux128_sb[:, dd * GC + c : dd * GC + c + 1]

        def b1v(dd, c):
            return aux128_sb[:, 32 + dd * GC + c : 32 + dd * GC + c + 1]

        def hwbv(w, c):
            return aux128_sb[:, 64 + 8 * w + c : 64 + 8 * w + c + 1]

        trans_sb = aux17_sb[:, 0:T]
        svec_sb = aux17_sb[:, T : T + 1]
        evec_sb = aux17_sb[:, T + 1 : T + 2]
        fcb_sb = aux17_sb[:, T + 2 : T + 3]
        oh_sb = aux17_sb[:, 20 : 20 + TOK].rearrange("t (b s) -> t b s", b=BL)
        cp_base = 20 + TOK
        s0e_sb = aux17_sb[:, cp_base + BL * T : cp_base + BL * T + 2 * BL]
        mren_sb = aux1_sb[:, 0 : NREN * BL].rearrange("o (k b) -> o k b", k=NREN)
        vm_sb = aux1_sb[:, NREN * BL :].rearrange("o (b s) -> o b s", b=BL)

        ones_t = pp.tile([T, 1], F32, tag="onesT")
        nc.vector.memset(ones_t[:], 1.0)
        ones_1t = pp.tile([1, T], F32, tag="ones1T")
        nc.vector.memset(ones_1t[:], 1.0)

        # ---- embedding gather + transpose --------------------------------------
        embX = pp.tile([128, TOK // 128, E], F32, tag="embX")
        for g in range(TOK // 128):
            nc.gpsimd.indirect_dma_start(
                out=embX[:, g, :],
                out_offset=None,
                in_=d["emb"][:],
                in_offset=bass.IndirectOffsetOnAxis(ap=x_sb[:, g : g + 1], axis=0),
            )
        embXbf = pp.tile([128, TOK // 128, E], BF16, tag="embXbf")
        for g in range(TOK // 128):
            nc.vector.tensor_copy(embXbf[:, g, :], embX[:, g, :])
        XT = pp.tile([128, E // 128, TOK], BF16, tag="XT")
        for k in range(E // 128):
            for g in range(TOK // 128):
                nc.sync.dma_start_transpose(
                    XT[:, k, bass.ts(g, 128)], embXbf[:, g, bass.ts(k, 128)]
                )

        # ---- L0 input GEMM ------------------------------------------------------
        wih0_sb = wp.tile([128, 2, 2, G], BF16, tag="wih")
        for dd in range(2):
            for k in range(2):
                dma(wih0_sb[:, dd, k, :], d["wih0T"][dd, k])
        gx = {}
        gx[("f", 0)] = pp.tile([128, GC, BL, S], BF16, tag="gxf", name="gx0f")
        gx[("b", 0)] = pp.tile([128, GC, BL, S], BF16, tag="gxb", name="gx0b")
        for dd, dn in enumerate("fb"):
            for c in range(GC):
                for b in range(BL):
                    pt = ps.tile([128, 256], F32, tag="mm")
                    for k in range(2):
                        nc.tensor.matmul(
                            pt[:],
                            wih0_sb[:, dd, k, bass.ts(c, 128)],
                            XT[:, k, bass.ts(b, 256)],
                            start=(k == 0),
                            stop=(k == 1),
                        )
                    nc.vector.tensor_scalar(
                        out=gx[(dn, 0)][:, c, b, :],
                        in0=pt[:],
                        scalar1=b0v(dd, c),
                        scalar2=None,
                        op0=ALU.add,
                    )

        # ---- recurrences --------------------------------------------------------
        whh_sb = {
            "f": wp.tile([128, 4, G], BF16, tag="whhf", name="whhf"),
            "b": wp.tile([128, 4, G], BF16, tag="whhb", name="whhb"),
        }
        for dd, dn in enumerate("fb"):
            for k in range(4):
                dma(whh_sb[dn][:, k, :], d["whh0T"][dd, k])

        hist = {}

        def lstm_layer(layer, steps):
            h_f = pp.tile([128, KH, S + 1, BL], BF16, tag="hhf")
            h_b = pp.tile([128, KH, S + 1, BL], BF16, tag="hhb")
            hist[(layer, "f")] = h_f
            hist[(layer, "b")] = h_b
            nc.vector.memset(h_f[:, :, 0, :], 0.0)
            nc.vector.memset(h_b[:, :, S, :], 0.0)
            cst = {}
            for dn in "fb":
                for par in range(2):
                    cst[(dn, par)] = pp.tile(
                        [128, KH, BL], F32, tag=f"c{dn}{par}", name=f"c{layer}{dn}{par}"
                    )
                nc.vector.memset(cst[(dn, 0)][:], 0.0)
            for t in range(steps):
                # per-dir chains; ifg/o PSUM split so the gate chain starts
                # while the same direction's o-gate matmuls are still running
                for dn, hh in (("f", h_f), ("b", h_b)):
                    pos = t if dn == "f" else S - 1 - t
                    rs = t if dn == "f" else pos + 1
                    ws = t + 1 if dn == "f" else pos
                    ptA = ps.tile([128, 12, BL], F32, tag=f"rec{dn}A", bufs=1)
                    ptB = ps.tile([128, 4, BL], F32, tag=f"rec{dn}B", bufs=1)
                    # emission order i,g,f,o: the c-state chain (i,g,f) finishes
                    # during the MMs; only the o-leg trails the last matmul
                    emit = [(c, ptA[:, c, :]) for c in range(4)]
                    emit += [(c, ptA[:, c - 4, :]) for c in range(8, 12)]
                    emit += [(c, ptA[:, c + 4, :]) for c in range(4, 8)]
                    emit += [(c, ptB[:, c - 12, :]) for c in range(12, 16)]
                    for c, dst in emit:
                        for k in range(KH):
                            nc.tensor.matmul(
                                dst,
                                whh_sb[dn][:, k, bass.ts(c, 128)],
                                hh[:, k, rs, :],
                                start=(k == 0),
                                stop=(k == KH - 1),
                            )
                    gxl = gx[(dn, layer)]
                    tmp = sp.tile([128, GC, BL], F32, tag=f"tmp{dn}")
                    sig = sp.tile([128, GC, BL], F32, tag=f"sig{dn}")
                    nc.vector.tensor_add(tmp[:, 0:4, :], ptA[:, 0:4, :], gxl[:, 0:4, :, pos])
                    nc.scalar.activation(sig[:, 0:4, :], tmp[:, 0:4, :], AF.Sigmoid)
                    nc.vector.tensor_add(tmp[:, 8:12, :], ptA[:, 4:8, :], gxl[:, 8:12, :, pos])
                    nc.scalar.activation(sig[:, 8:12, :], tmp[:, 8:12, :], AF.Tanh)
                    nc.vector.tensor_add(tmp[:, 4:8, :], ptA[:, 8:12, :], gxl[:, 4:8, :, pos])
                    nc.scalar.activation(sig[:, 4:8, :], tmp[:, 4:8, :], AF.Sigmoid)
                    c_old = cst[(dn, t % 2)]
                    c_new = cst[(dn, 1 - t % 2)]
                    ig = sp.tile([128, KH, BL], F32, tag=f"ig{dn}")
                    nc.vector.tensor_mul(ig[:], sig[:, 0:4, :], sig[:, 8:12, :])
                    nc.vector.tensor_mul(c_new[:], sig[:, 4:8, :], c_old[:])
                    nc.vector.tensor_add(c_new[:], c_new[:], ig[:])
                    th = sp.tile([128, KH, BL], F32, tag=f"th{dn}")
                    nc.scalar.activation(th[:], c_new[:], AF.Tanh)
                    nc.vector.tensor_add(
                        tmp[:, 12:16, :], ptB[:], gxl[:, 12:16, :, pos]
                    )
                    nc.scalar.activation(sig[:, 12:16, :], tmp[:, 12:16, :], AF.Sigmoid)
                    nc.vector.tensor_mul(hh[:, :, ws, :], sig[:, 12:16, :], th[:])

        lstm_layer(0, steps)

        # ---- L1 input GEMM ------------------------------------------------------
        gx[("f", 1)] = pp.tile([128, GC, BL, S], BF16, tag="gxf", name="gx1f")
        gx[("b", 1)] = pp.tile([128, GC, BL, S], BF16, tag="gxb", name="gx1b")
        for dd, dn in enumerate("fb"):
            wih1_sb = wp.tile([128, 8, G], BF16, tag="wih")
            for k in range(8):
                dma(wih1_sb[:, k, :], d["wih1T"][dd, k])
            for c in range(GC):
                for b in range(BL):
                    pt = ps.tile([128, 256], F32, tag="mm")
                    for k in range(8):
                        rhs = (
                            hist[(0, "f")][:, k, 1 : S + 1, b]
                            if k < KH
                            else hist[(0, "b")][:, k - KH, 0:S, b]
                        )
                        nc.tensor.matmul(
                            pt[:],
                            wih1_sb[:, k, bass.ts(c, 128)],
                            rhs,
                            start=(k == 0),
                            stop=(k == 7),
                        )
                    nc.vector.tensor_scalar(
                        out=gx[(dn, 1)][:, c, b, :],
                        in0=pt[:],
                        scalar1=b1v(dd, c),
                        scalar2=None,
                        op0=ALU.add,
                    )

        for dd, dn in enumerate("fb"):
            whh_sb[dn] = wp.tile([128, 4, G], BF16, tag=f"whh{dn}", name=f"whh1{dn}")
            for k in range(4):
                dma(whh_sb[dn][:, k, :], d["whh1T"][dd, k])
        lstm_layer(1, steps)

        # ---- highway + fc -------------------------------------------------------
        hw_sb = wp.tile([128, 2, 8, 2 * H], BF16, tag="wih")
        for w in range(2):
            for k in range(8):
                dma(hw_sb[:, w, k, :], d["hwT"][w, k])

        def x1_slice(k, b):
            if k < KH:
                return hist[(1, "f")][:, k, 1 : S + 1, b]
            return hist[(1, "b")][:, k - KH, 0:S, b]

        x2 = pp.tile([128, 8, TOK], BF16, tag="gxf")
        for c in range(8):
            for b in range(BL):
                ptt = ps.tile([128, 256], F32, tag="mm")
                pth = ps.tile([128, 256], F32, tag="mm")
                for k in range(8):
                    nc.tensor.matmul(
                        ptt[:], hw_sb[:, 0, k, bass.ts(c, 128)], x1_slice(k, b),
                        start=(k == 0), stop=(k == 7),
                    )
                for k in range(8):
                    nc.tensor.matmul(
                        pth[:], hw_sb[:, 1, k, bass.ts(c, 128)], x1_slice(k, b),
                        start=(k == 0), stop=(k == 7),
                    )
                tg = sp.tile([128, 256], F32, tag="tg")
                nc.scalar.activation(tg[:], ptt[:], AF.Sigmoid, bias=hwbv(0, c))
                rl = sp.tile([128, 256], F32, tag="rl")
                nc.scalar.activation(rl[:], pth[:], AF.Relu, bias=hwbv(1, c))
                dd_ = sp.tile([128, 256], F32, tag="dd")
                nc.vector.tensor_sub(dd_[:], rl[:], x1_slice(c, b))
                nc.vector.tensor_mul(dd_[:], tg[:], dd_[:])
                nc.vector.tensor_add(x2[:, c, bass.ts(b, 256)], dd_[:], x1_slice(c, b))

        logits = pp.tile([T, BL, S], F32, tag="embX")
        for b in range(BL):
            pt = ps.tile([128, 256], F32, tag="mm")
            for k in range(8):
                nc.tensor.matmul(
                    pt[:T, :], fcw_sb[:, k, :], x2[:, k, bass.ts(b, 256)],
                    start=(k == 0), stop=(k == 7),
                )
            nc.scalar.activation(logits[:, b, :], pt[:T, :], AF.Identity, bias=fcb_sb)

        # ---- CRF ---------------------------------------------------------------
        expEm = pp.tile([T, BL, S], F32, tag="XT")
        nc.scalar.activation(expEm[:], logits[:], AF.Exp)
        expT = pp.tile([T, T], F32, tag="expT")
        nc.scalar.activation(expT[:], trans_sb, AF.Exp)
        expS = pp.tile([T, 1], F32, tag="expS")
        nc.scalar.activation(expS[:], svec_sb, AF.Exp)
        expE = pp.tile([T, 1], F32, tag="expE")
        nc.scalar.activation(expE[:], evec_sb, AF.Exp)

        afin = pp.tile([T, BL], F32, tag="afin")
        lacc = {}
        for ch in range(2):
            for par in range(2):
                lacc[(ch, par)] = pp.tile(
                    [1, 2], F32, tag=f"lacc{ch}{par}", name=f"lacc{ch}{par}"
                )
            nc.vector.memset(lacc[(ch, 0)][:], 0.0)
        ap = ctx.enter_context(tc.tile_pool(name="crf", bufs=4))

        # two independent 2-sequence scan chains, interleaved per step
        A = {}
        for ch in range(2):
            sl = slice(2 * ch, 2 * ch + 2)
            A[ch] = ap.tile([T, 2], F32, tag=f"A{ch}", name=f"A{ch}")
            nc.vector.tensor_scalar(
                out=A[ch][:], in0=expEm[:, sl, 0], scalar1=expS[:, 0:1],
                scalar2=None, op0=ALU.mult,
            )
        nren_seen = 0
        for t in range(1, steps):
            for ch in range(2):
                sl = slice(2 * ch, 2 * ch + 2)
                pt = ps.tile([128, 2], F32, tag="mm")
                nc.tensor.matmul(pt[:T, :], expT[:], A[ch][:], start=True, stop=True)
                A[ch] = ap.tile([T, 2], F32, tag=f"A{ch}", name=f"A{ch}")
                nc.vector.tensor_mul(A[ch][:], pt[:T, :], expEm[:, sl, t])
            if t % RENORM == 0:
                for ch in range(2):
                    sl = slice(2 * ch, 2 * ch + 2)
                    psS = ps.tile([1, 512], F32, tag="small")
                    nc.tensor.matmul(
                        psS[:, :2], ones_t[:], A[ch][:], start=True, stop=True
                    )
                    Ssb = ap.tile([1, 2], F32, tag=f"Ssb{ch}", name=f"Ssb{ch}")
                    nc.vector.tensor_copy(Ssb[:], psS[:, :2])
                    Sr = ap.tile([1, 2], F32, tag=f"Sr{ch}", name=f"Sr{ch}")
                    nc.vector.reciprocal(Sr[:], Ssb[:])
                    pB = ps.tile([128, 2], F32, tag="mm")
                    nc.tensor.matmul(pB[:T, :], ones_1t[:], Sr[:], start=True, stop=True)
                    A2 = ap.tile([T, 2], F32, tag=f"A{ch}", name=f"A{ch}")
                    nc.vector.tensor_mul(A2[:], A[ch][:], pB[:T, :])
                    A[ch] = A2
                    lnS = ap.tile([1, 2], F32, tag=f"lnS{ch}", name=f"lnS{ch}")
                    nc.scalar.activation(lnS[:], Ssb[:], AF.Ln)
                    nc.vector.tensor_mul(lnS[:], lnS[:], mren_sb[:, nren_seen, sl])
                    old, new = lacc[(ch, nren_seen % 2)], lacc[(ch, 1 - nren_seen % 2)]
                    nc.vector.tensor_add(new[:], old[:], lnS[:])
                nren_seen += 1
            if t >= min(S // 2 - 1, steps - 1):
                for ch in range(2):
                    sl = slice(2 * ch, 2 * ch + 2)
                    nc.vector.copy_predicated(
                        afin[:, sl], msel_sb[:, sl, t], A[ch][:]
                    )

        # logZ = ln(sum_j afin*expE) + lacc
        ae = op.tile([T, BL], F32, tag="ae")
        nc.vector.tensor_scalar(
            out=ae[:], in0=afin[:], scalar1=expE[:, 0:1], scalar2=None, op0=ALU.mult
        )
        psZ = ps.tile([1, 512], F32, tag="small")
        nc.tensor.matmul(psZ[:, :BL], ones_t[:], ae[:], start=True, stop=True)
        logZ = sp.tile([1, BL], F32, tag="logZ")
        nc.scalar.activation(logZ[:], psZ[:, :BL], AF.Ln)
        for ch in range(2):
            sl = slice(2 * ch, 2 * ch + 2)
            nc.vector.tensor_add(
                logZ[:, sl], logZ[:, sl], lacc[(ch, nren_seen % 2)][:]
            )

        # ---- numerator ----------------------------------------------------------
        emm = op.tile([T, BL, S], F32, tag="emm")
        nc.vector.tensor_mul(emm[:], logits[:], oh_sb)
        empart = sp.tile([T, BL], F32, tag="empart")
        nc.vector.reduce_sum(empart[:], emm[:], axis=AX.X)
        nv = sp.tile([T, BL], F32, tag="nv")
        nc.vector.tensor_scalar(
            out=nv[:], in0=s0e_sb[:, 0:BL], scalar1=svec_sb, scalar2=None,
            op0=ALU.mult,
        )
        ev = sp.tile([T, BL], F32, tag="ev")
        nc.vector.tensor_scalar(
            out=ev[:], in0=s0e_sb[:, BL : 2 * BL], scalar1=evec_sb, scalar2=None,
            op0=ALU.mult,
        )
        nc.vector.tensor_add(nv[:], nv[:], ev[:])
        nc.vector.tensor_add(nv[:], nv[:], empart[:])
        for b in range(BL):
            trp = op.tile([T, T], F32, tag="trp")
            nc.vector.tensor_mul(trp[:], aux17_sb[:, cp_base + T * b : cp_base + T * (b + 1)], trans_sb)
            trr = sp.tile([T, 1], F32, tag="trr")
            nc.vector.reduce_sum(trr[:], trp[:], axis=AX.X)
            nc.vector.tensor_add(nv[:, b : b + 1], nv[:, b : b + 1], trr[:])
        psN = ps.tile([1, 512], F32, tag="small")
        nc.tensor.matmul(psN[:, :BL], ones_t[:], nv[:], start=True, stop=True)
        num_sb = sp.tile([1, BL], F32, tag="num")
        nc.vector.tensor_copy(num_sb[:], psN[:, :BL])

        # ---- aux CE -------------------------------------------------------------
        psE = ps.tile([1, 512], F32, tag="small")
        lse = op.tile([1, BL, S], F32, tag="lse")
        for hlf in range(2):
            nc.tensor.matmul(
                psE[:, :512],
                ones_t[:],
                expEm[:, 2 * hlf : 2 * hlf + 2, :],
                start=True,
                stop=True,
            )
            nc.scalar.activation(
                lse[:, 2 * hlf : 2 * hlf + 2, :],
                psE[:].rearrange("o (b s) -> o b s", b=2),
                AF.Ln,
            )
        nc.vector.tensor_mul(lse[:], lse[:], vm_sb)
        lsum = sp.tile([1, BL], F32, tag="lsum")
        nc.vector.reduce_sum(lsum[:], lse[:], axis=AX.X)
        psM = ps.tile([1, 512], F32, tag="small")
        nc.tensor.matmul(psM[:, :BL], ones_t[:], empart[:], start=True, stop=True)
        aux_sb = sp.tile([1, BL], F32, tag="aux")
        nc.vector.tensor_sub(aux_sb[:], lsum[:], psM[:, :BL])

        dma(out_d[0:1, :], num_sb[:])
        dma(out_d[1:2, :], logZ[:])
        dma(out_d[2:3, :], aux_sb[:])

    nc.compile()
    return nc


def _prep_maps(inputs):
    bf = ml_dtypes.bfloat16
    x = np.asarray(inputs["x"]).astype(np.int32)
    tags = np.asarray(inputs["tags"]).astype(np.int32)
    emb = np.asarray(inputs["emb"], np.float32)

    def t2(w):  # (2, G, K) -> (2, K//128, 128, G)
        w = np.asarray(w, np.float32)
        K = w.shape[2]
        return np.ascontiguousarray(
            w.transpose(0, 2, 1).reshape(2, K // 128, 128, G)
        ).astype(bf)

    wih0 = t2(inputs["w_ih_l0"])
    whh0 = t2(inputs["w_hh_l0"])
    wih1 = t2(inputs["w_ih_l1"])
    whh1 = t2(inputs["w_hh_l1"])
    b0 = np.asarray(inputs["b_l0"], np.float32).reshape(2, GC, 128)
    b1 = np.asarray(inputs["b_l1"], np.float32).reshape(2, GC, 128)
    aux128 = np.zeros((128, 80), np.float32)
    aux128[:, 0:32] = b0.reshape(32, 128).T
    aux128[:, 32:64] = b1.reshape(32, 128).T
    hwT = np.stack(
        [
            np.asarray(inputs["hw_t_w"], np.float32).T.reshape(8, 128, 2 * H),
            np.asarray(inputs["hw_h_w"], np.float32).T.reshape(8, 128, 2 * H),
        ]
    ).astype(bf)
    aux128[:, 64:72] = np.asarray(inputs["hw_t_b"], np.float32).reshape(8, 128).T
    aux128[:, 72:80] = np.asarray(inputs["hw_h_b"], np.float32).reshape(8, 128).T
    fcwT = np.ascontiguousarray(
        np.asarray(inputs["fc_w"], np.float32).T.reshape(8, 128, T).transpose(1, 0, 2)
    ).reshape(128, 8 * T).astype(bf)
    trans = np.asarray(inputs["crf_trans"], np.float32)
    svec = np.asarray(inputs["crf_start"], np.float32)
    evec = np.asarray(inputs["crf_end"], np.float32)
    fcb = np.asarray(inputs["fc_b"], np.float32)

    valid = tags != 0
    lengths = (x != 0).sum(1)

    maps = []
    for c in range(NC):
        sl = slice(c * BL, (c + 1) * BL)
        xl, tl, vl, ll = x[sl], tags[sl], valid[sl], lengths[sl]
        flat = xl.reshape(-1)  # 256*b + t
        x_idx = np.ascontiguousarray(flat.reshape(TOK // 128, 128).T).astype(np.int32)
        jj = np.arange(T)
        oh = (tl[None, :, :] == jj[:, None, None]) & vl[None, :, :]
        oh_tags = oh.reshape(T, TOK).astype(np.float32)
        cp = np.zeros((T, BL, T), np.float32)
        for b in range(BL):
            for t in range(1, S):
                if vl[b, t]:
                    cp[tl[b, t - 1], b, tl[b, t]] += 1.0
        s0e = np.zeros((T, 2 * BL), np.float32)
        for b in range(BL):
            s0e[tl[b, 0], b] = 1.0
            s0e[tl[b, ll[b] - 1], BL + b] = 1.0
        msel = np.zeros((BL, S), np.float32)
        for b in range(BL):
            msel[b, ll[b] - 1] = 1.0
        msel = np.broadcast_to(msel.reshape(1, TOK), (T, TOK)).astype(np.uint8)
        mren = np.zeros((NREN, BL), np.float32)
        for k in range(NREN):
            mren[k] = (RENORM * (k + 1) <= ll - 1).astype(np.float32)
        aux17 = np.zeros((T, 1120), np.float32)
        aux17[:, 0:T] = trans
        aux17[:, T] = svec
        aux17[:, T + 1] = evec
        aux17[:, T + 2] = fcb
        aux17[:, 20 : 20 + TOK] = oh_tags
        aux17[:, 20 + TOK : 20 + TOK + BL * T] = cp.transpose(0, 1, 2).reshape(T, BL * T)
        aux17[:, 20 + TOK + BL * T : 20 + TOK + BL * T + 2 * BL] = s0e
        aux1 = np.concatenate(
            [mren.reshape(-1), vl.reshape(-1).astype(np.float32)]
        ).reshape(1, -1)
        maps.append(
            dict(
                x_idx=x_idx,
                emb=emb,
                wih0T=wih0,
                whh0T=whh0,
                wih1T=wih1,
                whh1T=whh1,
                hwT=hwT,
                fcwT=fcwT,
                aux128=aux128,
                aux17=aux17,
                aux1=aux1,
                msel=msel,
            )
        )
    return maps, valid


TRACE = {}


def kernel(**inputs):
    if "nc" not in _CACHE:
        _CACHE["nc"] = _build_nc()
    nc = _CACHE["nc"]
    maps, valid = _prep_maps(inputs)
    kw = {}
    if TRACE.get("on"):
        kw = dict(trace=True, tmpdir=TRACE.get("dir"), trace_cores=[0])
    res = run_bass_kernel_spmd(nc, maps, list(range(NC)), **kw)
    TRACE["last"] = res
    outs = [res.results[i]["out"] for i in range(NC)]
    num = np.concatenate([o[0] for o in outs])
    logZ = np.concatenate([o[1] for o in outs])
    aux = np.concatenate([o[2] for o in outs])
    crf_loss = -np.mean(num - logZ, dtype=np.float32)
    aux_loss = np.float32(aux.sum()) / np.float32(max(valid.sum(), 1))
    return np.float32(crf_loss + np.float32(0.1) * aux_loss)



# revision 3
# speedup vs baseline: 1.0272x; 1.0272x over previous
"""BiLSTM-CRF forward loss on 8 TRN2 NeuronCores (Bass/Tile), v2.

Structure (v2 "pair plan"): the LSTM recurrence is LDWEIGHTS-bound on the
tensor engine (weight-load cost is independent of batch), so instead of the
v1 layout (every core runs both directions for 4 sequences) each core runs
ONE direction for 8 sequences. Direction is encoded purely in per-core data
(backward cores receive time-reversed token sequences and k-permuted weight
chunks), keeping the SPMD instruction stream identical across cores.
Paired cores (c, c+4) exchange hidden-state histories between layers via
pairwise AllGather collectives; a mirrored (time-reversed) copy of h is
maintained during the recurrence so each side receives the partner history
already in its own traversal order. Recurrent weights are fp8 (x32 scaled,
scale folded out via the activation `scale` argument), which roughly halves
LDWEIGHTS time versus bf16.

Forward cores (0-3) produce CRF logZ + numerator + aux-CE for their 8
sequences in natural order; backward cores' tail outputs are ignored.
"""
import sys

import numpy as np

try:
    import concourse  # noqa: F401
except ImportError:  # pragma: no cover
    sys.path.insert(0, "/opt/trn_rl_repo")

import ml_dtypes
from contextlib import ExitStack

import concourse.bass as bass
import concourse.bacc as bacc
import concourse.mybir as mybir
import concourse.tile as tile
from concourse.bass_utils import run_bass_kernel_spmd

F32 = mybir.dt.float32
BF16 = mybir.dt.bfloat16
FP8 = mybir.dt.float8e4
U8 = mybir.dt.uint8
I32 = mybir.dt.int32
AF = mybir.ActivationFunctionType
ALU = mybir.AluOpType
AX = mybir.AxisListType

B, S, E, H, T, V = 32, 256, 256, 512, 17, 50000
NC = 8
NSEQ = 8              # sequences per pair-group (one direction per core)
TOK = NSEQ * S        # 2048 local tokens, flat = 256*q + s (q=seq, s=step)
G = 4 * H             # 2048 gate rows
GC = G // 128         # 16 gate chunks
KH = H // 128         # 4 hidden chunks
KE = E // 128         # 2 embedding chunks
K1 = 2 * H // 128     # 8 L1-input chunks
RENORM = 8
NREN = (S - 1) // RENORM   # 31
SPL = 64              # exchange split: slots [SPL:S] shipped early
USE_FP8 = True
WSCALE = 32.0 if USE_FP8 else 1.0
WDT = FP8 if USE_FP8 else BF16

_CACHE = {}


def _build_nc():
    nc = bacc.Bacc(None, target_bir_lowering=False, num_devices=NC)
    d = {}
    P = nc.declare_dram_parameter
    d["x_idx"] = P("x_idx", [128, TOK // 128], I32, isOutput=False)
    d["emb"] = P("emb", [V, E], F32, isOutput=False)
    d["wih0T"] = P("wih0T", [KE, 128, G], BF16, isOutput=False)
    d["whh0T"] = P("whh0T", [KH, 128, G], WDT, isOutput=False)
    d["wih1T"] = P("wih1T", [K1, 128, G], BF16, isOutput=False)
    d["whh1T"] = P("whh1T", [KH, 128, G], WDT, isOutput=False)
    d["hwT"] = P("hwT", [2, K1, 128, 2 * H], BF16, isOutput=False)
    d["fcwT"] = P("fcwT", [128, K1 * T], BF16, isOutput=False)
    d["aux128"] = P("aux128", [128, 48], F32, isOutput=False)
    d["aux17"] = P("aux17", [T, 2220], F32, isOutput=False)
    d["mren"] = P("mren", [1, NREN * NSEQ], F32, isOutput=False)
    d["vm"] = P("vm", [1, TOK], U8, isOutput=False)
    d["msel"] = P("msel", [T, TOK], U8, isOutput=False)
    d["prow"] = P("prow", [128, 1], I32, isOutput=False)
    out_d = P("out", [4, NSEQ], F32, isOutput=True)

    with tile.TileContext(nc) as tc, ExitStack() as ctx:
        pp = ctx.enter_context(tc.tile_pool(name="persist", bufs=1))
        wp = ctx.enter_context(tc.tile_pool(name="wts", bufs=1))
        sp = ctx.enter_context(tc.tile_pool(name="small", bufs=2))
        op = ctx.enter_context(tc.tile_pool(name="once", bufs=1))
        ps = ctx.enter_context(tc.tile_pool(name="psum", bufs=2, space="PSUM"))
        dp = ctx.enter_context(tc.tile_pool(name="dram", bufs=1, space="DRAM"))

        dma = nc.sync.dma_start

        # ---- static small loads ------------------------------------------------
        x_sb = pp.tile([128, TOK // 128], I32, tag="xidx")
        dma(x_sb[:], d["x_idx"][:])
        fcw_sb = pp.tile([128, K1, T], BF16, tag="fcw")
        dma(fcw_sb[:], d["fcwT"][:].rearrange("p (k t) -> p k t", k=K1))
        aux128_sb = pp.tile([128, 48], F32, tag="aux128")
        dma(aux128_sb[:], d["aux128"][:])
        aux17_sb = pp.tile([T, 2220], F32, tag="aux17")
        dma(aux17_sb[:], d["aux17"][:])
        mren_sb = pp.tile([1, NREN, NSEQ], F32, tag="mren")
        dma(mren_sb[:], d["mren"][:].rearrange("o (k b) -> o k b", k=NREN))
        vm_sb = pp.tile([1, NSEQ, S], U8, tag="vm")
        dma(vm_sb[:], d["vm"][:].rearrange("o (b s) -> o b s", b=NSEQ))
        msel_sb = pp.tile([T, NSEQ, S], U8, tag="msel")
        dma(msel_sb[:], d["msel"][:].rearrange("t (b s) -> t b s", b=NSEQ))
        prow_sb = pp.tile([128, 1], I32, tag="prow")
        dma(prow_sb[:], d["prow"][:])

        def b0v(c):
            return aux128_sb[:, c : c + 1]

        def b1v(c):
            return aux128_sb[:, 16 + c : 16 + c + 1]

        def hwbv(w, c):
            return aux128_sb[:, 32 + 8 * w + c : 32 + 8 * w + c + 1]

        trans_sb = aux17_sb[:, 0:T]
        svec_sb = aux17_sb[:, T : T + 1]
        evec_sb = aux17_sb[:, T + 1 : T + 2]
        fcb_sb = aux17_sb[:, T + 2 : T + 3]
        oh_sb = aux17_sb[:, 20 : 20 + TOK].rearrange("t (b s) -> t b s", b=NSEQ)
        cp_base = 20 + TOK
        s0e_sb = aux17_sb[:, cp_base + NSEQ * T : cp_base + NSEQ * T + 2 * NSEQ]

        ones_t = pp.tile([T, 1], F32, tag="onesT")
        nc.vector.memset(ones_t[:], 1.0)
        ones_1t = pp.tile([1, T], F32, tag="ones1T")
        nc.vector.memset(ones_1t[:], 1.0)

        # ---- embedding gather + transpose --------------------------------------
        XT = pp.tile([128, KE, TOK], BF16, tag="XT")
        for half in range(4):
            embX = sp.tile([128, 4, E], F32, tag="embX")
            for g in range(4):
                nc.gpsimd.indirect_dma_start(
                    out=embX[:, g, :],
                    out_offset=None,
                    in_=d["emb"][:],
                    in_offset=bass.IndirectOffsetOnAxis(
                        ap=x_sb[:, 4 * half + g : 4 * half + g + 1], axis=0
                    ),
                )
            embXbf = sp.tile([128, 4, E], BF16, tag="embXb")
            nc.vector.tensor_copy(embXbf[:], embX[:])
            for k in range(KE):
                for g in range(4):
                    nc.sync.dma_start_transpose(
                        XT[:, k, bass.ts(4 * half + g, 128)],
                        embXbf[:, g, bass.ts(k, 128)],
                    )

        # ---- L0 input GEMM -----------------------------------------------------
        wih_sb = wp.tile([128, K1, G], BF16, tag="wih")
        for k in range(KE):
            dma(wih_sb[:, k, :], d["wih0T"][k])
        gx = pp.tile([128, GC, NSEQ, S], BF16, tag="gx", name="gx0")
        for c in range(GC):
            for b in range(NSEQ):
                pt = ps.tile([128, 256], F32, tag="mm")
                for k in range(KE):
                    nc.tensor.matmul(
                        pt[:],
                        wih_sb[:, k, bass.ts(c, 128)],
                        XT[:, k, bass.ts(b, 256)],
                        start=(k == 0),
                        stop=(k == KE - 1),
                    )
                nc.vector.tensor_scalar(
                    out=gx[:, c, b, :], in0=pt[:], scalar1=b0v(c), scalar2=None,
                    op0=ALU.add,
                )

        # ---- recurrence + exchange --------------------------------------------
        SZA = (S - SPL) * KH * NSEQ
        SZB = SPL * KH * NSEQ
        RG = [[0, 4], [1, 5], [2, 6], [3, 7]]

        def lstm_layer(layer, whh_sb, gxl):
            hist = pp.tile(
                [128, KH, S + 1, NSEQ], BF16, tag="hist", name=f"hist{layer}"
            )
            mir = pp.tile([128, S, KH, NSEQ], BF16, tag="mir", name=f"mir{layer}")
            bAi = dp.tile([128, SZA], BF16, tag="bAi", name=f"bAi{layer}")
            bAo = dp.tile([256, SZA], BF16, tag="bAo", name=f"bAo{layer}")
            bBi = dp.tile([128, SZB], BF16, tag="bBi", name=f"bBi{layer}")
            bBo = dp.tile([256, SZB], BF16, tag="bBo", name=f"bBo{layer}")
            nc.vector.memset(hist[:, :, 0, :], 0.0)
            cst = {}
            for par in range(2):
                cst[par] = pp.tile(
                    [128, KH, NSEQ], F32, tag=f"cst{par}", name=f"c{layer}p{par}"
                )
            nc.vector.memset(cst[0][:], 0.0)
            for t in range(S):
                if t == 192:
                    # slots [SPL:S] hold h of steps 0..S-1-SPL (done) — ship now
                    dma(bAi[:], mir[:, SPL:S].rearrange("p s k q -> p (s k q)"))
                    nc.gpsimd.collective_compute(
                        "AllGather", ALU.bypass, replica_groups=RG,
                        ins=[bAi[:].opt()], outs=[bAo[:].opt()],
                    )
                ptA = ps.tile([128, 12, NSEQ], F32, tag="recA", bufs=1)
                ptB = ps.tile([128, 4, NSEQ], F32, tag="recB", bufs=1)
                emit = [(c, ptA[:, c, :]) for c in range(4)]
                emit += [(c, ptA[:, c - 4, :]) for c in range(8, 12)]
                emit += [(c, ptA[:, c + 4, :]) for c in range(4, 8)]
                emit += [(c, ptB[:, c - 12, :]) for c in range(12, 16)]
                for c, dst in emit:
                    for k in range(KH):
                        nc.tensor.matmul(
                            dst,
                            whh_sb[:, k, bass.ts(c, 128)],
                            hist[:, k, t, :],
                            start=(k == 0),
                            stop=(k == KH - 1),
                        )
                tmp = sp.tile([128, GC, NSEQ], F32, tag="tmp")
                sig = sp.tile([128, GC, NSEQ], F32, tag="sig")
                nc.vector.tensor_add(tmp[:, 0:4, :], ptA[:, 0:4, :], gxl[:, 0:4, :, t])
                nc.scalar.activation(
                    sig[:, 0:4, :], tmp[:, 0:4, :], AF.Sigmoid, scale=1.0 / WSCALE
                )
                nc.vector.tensor_add(tmp[:, 8:12, :], ptA[:, 4:8, :], gxl[:, 8:12, :, t])
                nc.scalar.activation(
                    sig[:, 8:12, :], tmp[:, 8:12, :], AF.Tanh, scale=1.0 / WSCALE
                )
                nc.vector.tensor_add(tmp[:, 4:8, :], ptA[:, 8:12, :], gxl[:, 4:8, :, t])
                nc.scalar.activation(
                    sig[:, 4:8, :], tmp[:, 4:8, :], AF.Sigmoid, scale=1.0 / WSCALE
                )
                c_old, c_new = cst[t % 2], cst[1 - t % 2]
                ig = sp.tile([128, KH, NSEQ], F32, tag="ig")
                nc.vector.tensor_mul(ig[:], sig[:, 0:4, :], sig[:, 8:12, :])
                nc.vector.tensor_mul(c_new[:], sig[:, 4:8, :], c_old[:])
                nc.vector.tensor_add(c_new[:], c_new[:], ig[:])
                th = sp.tile([128, KH, NSEQ], F32, tag="th")
                nc.scalar.activation(th[:], c_new[:], AF.Tanh)
                # o-gate split in halves so next step's early k-chunks unblock
                for ho in range(2):
                    cs = slice(12 + 2 * ho, 14 + 2 * ho)
                    ks = slice(2 * ho, 2 * ho + 2)
                    nc.vector.tensor_add(
                        tmp[:, cs, :], ptB[:, 2 * ho : 2 * ho + 2, :], gxl[:, cs, :, t]
                    )
                    nc.scalar.activation(
                        sig[:, cs, :], tmp[:, cs, :], AF.Sigmoid, scale=1.0 / WSCALE
                    )
                    nc.vector.tensor_mul(
                        hist[:, ks, t + 1, :], sig[:, cs, :], th[:, ks, :]
                    )
                    nc.vector.tensor_copy(
                        mir[:, S - 1 - t, ks, :], hist[:, ks, t + 1, :]
                    )
            dma(bBi[:], mir[:, 0:SPL].rearrange("p s k q -> p (s k q)"))
            nc.gpsimd.collective_compute(
                "AllGather", ALU.bypass, replica_groups=RG,
                ins=[bBi[:].opt()], outs=[bBo[:].opt()],
            )
            part = pp.tile([128, S, KH, NSEQ], BF16, tag="part", name=f"part{layer}")
            nc.gpsimd.indirect_dma_start(
                out=part[:, SPL:S].rearrange("p s k q -> p (s k q)"),
                out_offset=None,
                in_=bAo[:],
                in_offset=bass.IndirectOffsetOnAxis(ap=prow_sb[:, 0:1], axis=0),
            )
            nc.gpsimd.indirect_dma_start(
                out=part[:, 0:SPL].rearrange("p s k q -> p (s k q)"),
                out_offset=None,
                in_=bBo[:],
                in_offset=bass.IndirectOffsetOnAxis(ap=prow_sb[:, 0:1], axis=0),
            )
            return hist, part

        whh_sb = wp.tile([128, KH, G], WDT, tag="whh", name="whh0")
        for k in range(KH):
            dma(whh_sb[:, k, :], d["whh0T"][k])
        hist0, part0 = lstm_layer(0, whh_sb, gx)

        # ---- L1 input GEMM -----------------------------------------------------
        wih_sb = wp.tile([128, K1, G], BF16, tag="wih", name="wih1")
        for k in range(K1):
            dma(wih_sb[:, k, :], d["wih1T"][k])

        def x_slice(hist, part, k, b):
            if k < KH:
                return hist[:, k, 1 : S + 1, b]
            return part[:, :, k - KH, b]

        gx1 = pp.tile([128, GC, NSEQ, S], BF16, tag="gx", name="gx1")
        for c in range(GC):
            for b in range(NSEQ):
                pt = ps.tile([128, 256], F32, tag="mm")
                for k in range(K1):
                    nc.tensor.matmul(
                        pt[:],
                        wih_sb[:, k, bass.ts(c, 128)],
                        x_slice(hist0, part0, k, b),
                        start=(k == 0),
                        stop=(k == K1 - 1),
                    )
                nc.vector.tensor_scalar(
                    out=gx1[:, c, b, :], in0=pt[:], scalar1=b1v(c), scalar2=None,
                    op0=ALU.add,
                )

        whh_sb = wp.tile([128, KH, G], WDT, tag="whh", name="whh1")
        for k in range(KH):
            dma(whh_sb[:, k, :], d["whh1T"][k])
        hist1, part1 = lstm_layer(1, whh_sb, gx1)

        # ---- highway + fc ------------------------------------------------------
        hw_sb = pp.tile([128, 2, K1, 2 * H], BF16, tag="gx", name="hw")
        for w in range(2):
            for k in range(K1):
                dma(hw_sb[:, w, k, :], d["hwT"][w, k])
        x2 = wp.tile([128, K1, TOK], BF16, tag="wih", name="x2")
        for c in range(8):
            for b in range(NSEQ):
                ptt = ps.tile([128, 256], F32, tag="mm")
                pth = ps.tile([128, 256], F32, tag="mm")
                for k in range(K1):
                    nc.tensor.matmul(
                        ptt[:], hw_sb[:, 0, k, bass.ts(c, 128)],
                        x_slice(hist1, part1, k, b),
                        start=(k == 0), stop=(k == K1 - 1),
                    )
                for k in range(K1):
                    nc.tensor.matmul(
                        pth[:], hw_sb[:, 1, k, bass.ts(c, 128)],
                        x_slice(hist1, part1, k, b),
                        start=(k == 0), stop=(k == K1 - 1),
                    )
                tg = sp.tile([128, 256], F32, tag="tg")
                nc.scalar.activation(tg[:], ptt[:], AF.Sigmoid, bias=hwbv(0, c))
                rl = sp.tile([128, 256], F32, tag="rl")
                nc.scalar.activation(rl[:], pth[:], AF.Relu, bias=hwbv(1, c))
                dd_ = sp.tile([128, 256], F32, tag="dd")
                nc.vector.tensor_sub(dd_[:], rl[:], x_slice(hist1, part1, c, b))
                nc.vector.tensor_mul(dd_[:], tg[:], dd_[:])
                nc.vector.tensor_add(
                    x2[:, c, bass.ts(b, 256)], dd_[:], x_slice(hist1, part1, c, b)
                )

        logits = op.tile([T, NSEQ, S], F32, tag="logits")
        for b in range(NSEQ):
            pt = ps.tile([128, 256], F32, tag="mm")
            for k in range(K1):
                nc.tensor.matmul(
                    pt[:T, :], fcw_sb[:, k, :], x2[:, k, bass.ts(b, 256)],
                    start=(k == 0), stop=(k == K1 - 1),
                )
            nc.scalar.activation(logits[:, b, :], pt[:T, :], AF.Identity, bias=fcb_sb)

        # ---- CRF ---------------------------------------------------------------
        expEm = pp.tile([T, NSEQ, S], F32, tag="XT", name="expEm")
        nc.scalar.activation(expEm[:], logits[:], AF.Exp)
        expT = op.tile([T, T], F32, tag="expT")
        nc.scalar.activation(expT[:], trans_sb, AF.Exp)
        expS = op.tile([T, 1], F32, tag="expS")
        nc.scalar.activation(expS[:], svec_sb, AF.Exp)
        expE = op.tile([T, 1], F32, tag="expE")
        nc.scalar.activation(expE[:], evec_sb, AF.Exp)

        afin = pp.tile([T, NSEQ], F32, tag="afin")
        lacc = {}
        for ch in range(2):
            for par in range(2):
                lacc[(ch, par)] = pp.tile(
                    [1, 4], F32, tag=f"lacc{ch}{par}", name=f"lacc{ch}{par}"
                )
            nc.vector.memset(lacc[(ch, 0)][:], 0.0)
        ap = ctx.enter_context(tc.tile_pool(name="crf", bufs=4))

        A = {}
        for ch in range(2):
            sl = slice(4 * ch, 4 * ch + 4)
            A[ch] = ap.tile([T, 4], F32, tag=f"A{ch}", name=f"A{ch}")
            nc.vector.tensor_scalar(
                out=A[ch][:], in0=expEm[:, sl, 0], scalar1=expS[:, 0:1],
                scalar2=None, op0=ALU.mult,
            )
        nren_seen = 0
        for t in range(1, S):
            for ch in range(2):
                sl = slice(4 * ch, 4 * ch + 4)
                pt = ps.tile([128, 4], F32, tag="mm")
                nc.tensor.matmul(pt[:T, :], expT[:], A[ch][:], start=True, stop=True)
                A[ch] = ap.tile([T, 4], F32, tag=f"A{ch}", name=f"A{ch}")
                nc.vector.tensor_mul(A[ch][:], pt[:T, :], expEm[:, sl, t])
            if t % RENORM == 0:
                for ch in range(2):
                    sl = slice(4 * ch, 4 * ch + 4)
                    psS = ps.tile([1, 512], F32, tag="small")
                    nc.tensor.matmul(
                        psS[:, :4], ones_t[:], A[ch][:], start=True, stop=True
                    )
                    Ssb = ap.tile([1, 4], F32, tag=f"Ssb{ch}", name=f"Ssb{ch}")
                    nc.vector.tensor_copy(Ssb[:], psS[:, :4])
                    Sr = ap.tile([1, 4], F32, tag=f"Sr{ch}", name=f"Sr{ch}")
                    nc.vector.reciprocal(Sr[:], Ssb[:])
                    pB = ps.tile([128, 4], F32, tag="mm")
                    nc.tensor.matmul(pB[:T, :], ones_1t[:], Sr[:], start=True, stop=True)
                    A2 = ap.tile([T, 4], F32, tag=f"A{ch}", name=f"A{ch}")
                    nc.vector.tensor_mul(A2[:], A[ch][:], pB[:T, :])
                    A[ch] = A2
                    lnS = ap.tile([1, 4], F32, tag=f"lnS{ch}", name=f"lnS{ch}")
                    nc.scalar.activation(lnS[:], Ssb[:], AF.Ln)
                    nc.vector.tensor_mul(lnS[:], lnS[:], mren_sb[:, nren_seen, sl])
                    old, new = lacc[(ch, nren_seen % 2)], lacc[(ch, 1 - nren_seen % 2)]
                    nc.vector.tensor_add(new[:], old[:], lnS[:])
                nren_seen += 1
            if t >= S // 2 - 1:
                for ch in range(2):
                    sl = slice(4 * ch, 4 * ch + 4)
                    nc.vector.copy_predicated(
                        afin[:, sl], msel_sb[:, sl, t], A[ch][:]
                    )

        ae = op.tile([T, NSEQ], F32, tag="ae")
        nc.vector.tensor_scalar(
            out=ae[:], in0=afin[:], scalar1=expE[:, 0:1], scalar2=None, op0=ALU.mult
        )
        psZ = ps.tile([1, 512], F32, tag="small")
        nc.tensor.matmul(psZ[:, :NSEQ], ones_t[:], ae[:], start=True, stop=True)
        logZ = sp.tile([1, NSEQ], F32, tag="logZ")
        nc.scalar.activation(logZ[:], psZ[:, :NSEQ], AF.Ln)
        for ch in range(2):
            sl = slice(4 * ch, 4 * ch + 4)
            nc.vector.tensor_add(
                logZ[:, sl], logZ[:, sl], lacc[(ch, nren_seen % 2)][:]
            )

        # ---- numerator ---------------------------------------------------------
        emm = pp.tile([T, NSEQ, S], F32, tag="mir", name="emm")
        nc.vector.tensor_mul(emm[:], logits[:], oh_sb)
        empart = sp.tile([T, NSEQ], F32, tag="empart")
        nc.vector.reduce_sum(empart[:], emm[:], axis=AX.X)
        nv = sp.tile([T, NSEQ], F32, tag="nv")
        nc.vector.tensor_scalar(
            out=nv[:], in0=s0e_sb[:, 0:NSEQ], scalar1=svec_sb, scalar2=None,
            op0=ALU.mult,
        )
        ev = sp.tile([T, NSEQ], F32, tag="ev")
        nc.vector.tensor_scalar(
            out=ev[:], in0=s0e_sb[:, NSEQ : 2 * NSEQ], scalar1=evec_sb, scalar2=None,
            op0=ALU.mult,
        )
        nc.vector.tensor_add(nv[:], nv[:], ev[:])
        nc.vector.tensor_add(nv[:], nv[:], empart[:])
        for b in range(NSEQ):
            trp = op.tile([T, T], F32, tag="trp")
            nc.vector.tensor_mul(
                trp[:], aux17_sb[:, cp_base + T * b : cp_base + T * (b + 1)], trans_sb
            )
            trr = sp.tile([T, 1], F32, tag="trr")
            nc.vector.reduce_sum(trr[:], trp[:], axis=AX.X)
            nc.vector.tensor_add(nv[:, b : b + 1], nv[:, b : b + 1], trr[:])
        psN = ps.tile([1, 512], F32, tag="small")
        nc.tensor.matmul(psN[:, :NSEQ], ones_t[:], nv[:], start=True, stop=True)
        num_sb = sp.tile([1, NSEQ], F32, tag="num")
        nc.vector.tensor_copy(num_sb[:], psN[:, :NSEQ])

        # ---- aux CE ------------------------------------------------------------
        lse = pp.tile([1, NSEQ, S], F32, tag="part", name="lse")
        for hlf in range(4):
            psE = ps.tile([1, 512], F32, tag="small")
            nc.tensor.matmul(
                psE[:, :512],
                ones_t[:],
                expEm[:, 2 * hlf : 2 * hlf + 2, :],
                start=True,
                stop=True,
            )
            nc.scalar.activation(
                lse[:, 2 * hlf : 2 * hlf + 2, :],
                psE[:].rearrange("o (b s) -> o b s", b=2),
                AF.Ln,
            )
        nc.vector.tensor_mul(lse[:], lse[:], vm_sb)
        lsum = sp.tile([1, NSEQ], F32, tag="lsum")
        nc.vector.reduce_sum(lsum[:], lse[:], axis=AX.X)
        psM = ps.tile([1, 512], F32, tag="small")
        nc.tensor.matmul(psM[:, :NSEQ], ones_t[:], empart[:], start=True, stop=True)
        aux_sb = sp.tile([1, NSEQ], F32, tag="aux")
        nc.vector.tensor_sub(aux_sb[:], lsum[:], psM[:, :NSEQ])

        dma(out_d[0:1, :], num_sb[:])
        dma(out_d[1:2, :], logZ[:])
        dma(out_d[2:3, :], aux_sb[:])

    nc.compile()
    return nc


def _prep_maps(inputs):
    bf = ml_dtypes.bfloat16
    f8 = ml_dtypes.float8_e4m3
    x = np.asarray(inputs["x"]).astype(np.int32)
    tags = np.asarray(inputs["tags"]).astype(np.int32)
    emb = np.asarray(inputs["emb"], np.float32)
    sc = np.float32(WSCALE)

    def t_chunks(w, perm=None):
        # (G, K) -> (K//128, 128, G), optional input-chunk permutation
        w = np.asarray(w, np.float32)
        K = w.shape[1]
        wT = np.ascontiguousarray(w.T).reshape(K // 128, 128, G)
        if perm is not None:
            wT = wT[perm]
        return wT

    wih0 = [t_chunks(sc * np.asarray(inputs["w_ih_l0"], np.float32)[dd]) for dd in range(2)]
    whh0 = [t_chunks(sc * np.asarray(inputs["w_hh_l0"], np.float32)[dd]) for dd in range(2)]
    whh1 = [t_chunks(sc * np.asarray(inputs["w_hh_l1"], np.float32)[dd]) for dd in range(2)]
    swap = [4, 5, 6, 7, 0, 1, 2, 3]
    wih1 = {}
    for cls in range(2):  # 0 = fwd core, 1 = bwd core
        perm = None if cls == 0 else swap
        wih1[cls] = [
            t_chunks(sc * np.asarray(inputs["w_ih_l1"], np.float32)[dd], perm)
            for dd in range(2)
        ]

    PH = np.arange(2 * H)
    PH_swap = np.concatenate([PH[H:], PH[:H]])
    hw_t = np.asarray(inputs["hw_t_w"], np.float32)
    hw_h = np.asarray(inputs["hw_h_w"], np.float32)
    hw_tb = np.asarray(inputs["hw_t_b"], np.float32)
    hw_hb = np.asarray(inputs["hw_h_b"], np.float32)
    fcw = np.asarray(inputs["fc_w"], np.float32)
    hwT, fcwT, hwb = {}, {}, {}
    for cls in range(2):
        pr = PH if cls == 0 else PH_swap
        ht = hw_t[np.ix_(pr, pr)]
        hh = hw_h[np.ix_(pr, pr)]
        hwT[cls] = np.stack(
            [ht.T.reshape(K1, 128, 2 * H), hh.T.reshape(K1, 128, 2 * H)]
        ).astype(bf)
        fcwT[cls] = (
            np.ascontiguousarray(fcw[:, pr].T)
            .reshape(K1, 128, T)
            .transpose(1, 0, 2)
            .reshape(128, K1 * T)
            .astype(bf)
        )
        hwb[cls] = (
            hw_tb[pr].reshape(8, 128).T,
            hw_hb[pr].reshape(8, 128).T,
        )

    b0 = sc * np.asarray(inputs["b_l0"], np.float32).reshape(2, GC, 128)
    b1 = sc * np.asarray(inputs["b_l1"], np.float32).reshape(2, GC, 128)
    trans = np.asarray(inputs["crf_trans"], np.float32)
    svec = np.asarray(inputs["crf_start"], np.float32)
    evec = np.asarray(inputs["crf_end"], np.float32)
    fcb = np.asarray(inputs["fc_b"], np.float32)

    valid = tags != 0
    lengths = (x != 0).sum(1)

    maps = []
    for core in range(NC):
        cls = 0 if core < 4 else 1  # fwd / bwd
        dd = cls
        g = core % 4
        sl = slice(g * NSEQ, (g + 1) * NSEQ)
        xl_nat, tl, vl, ll = x[sl], tags[sl], valid[sl], lengths[sl]
        xl = xl_nat if cls == 0 else xl_nat[:, ::-1]
        flat = np.ascontiguousarray(xl).reshape(-1)
        x_idx = np.ascontiguousarray(flat.reshape(TOK // 128, 128).T).astype(np.int32)

        aux128 = np.zeros((128, 48), np.float32)
        aux128[:, 0:16] = b0[dd].reshape(16, 128).T
        aux128[:, 16:32] = b1[dd].reshape(16, 128).T
        aux128[:, 32:40] = hwb[cls][0]
        aux128[:, 40:48] = hwb[cls][1]

        # CRF / numerator / aux masks: natural order, used only on fwd cores
        jj = np.arange(T)
        oh = (tl[None, :, :] == jj[:, None, None]) & vl[None, :, :]
        oh_tags = oh.reshape(T, TOK).astype(np.float32)
        cp = np.zeros((T, NSEQ, T), np.float32)
        prev, cur = tl[:, :-1], tl[:, 1:]
        vstep = vl[:, 1:]
        for b in range(NSEQ):
            np.add.at(cp[:, b, :], (prev[b][vstep[b]], cur[b][vstep[b]]), 1.0)
        s0e = np.zeros((T, 2 * NSEQ), np.float32)
        for b in range(NSEQ):
            s0e[tl[b, 0], b] = 1.0
            s0e[tl[b, ll[b] - 1], NSEQ + b] = 1.0
        msel = np.zeros((NSEQ, S), np.float32)
        for b in range(NSEQ):
            msel[b, ll[b] - 1] = 1.0
        msel = np.broadcast_to(msel.reshape(1, TOK), (T, TOK)).astype(np.uint8)
        mren = np.zeros((NREN, NSEQ), np.float32)
        for k in range(NREN):
            mren[k] = (RENORM * (k + 1) <= ll - 1).astype(np.float32)
        aux17 = np.zeros((T, 2220), np.float32)
        aux17[:, 0:T] = trans
        aux17[:, T] = svec
        aux17[:, T + 1] = evec
        aux17[:, T + 2] = fcb
        aux17[:, 20 : 20 + TOK] = oh_tags
        aux17[:, 20 + TOK : 20 + TOK + NSEQ * T] = cp.reshape(T, NSEQ * T)
        aux17[:, 20 + TOK + NSEQ * T : 20 + TOK + NSEQ * T + 2 * NSEQ] = s0e
        vm = vl.reshape(1, TOK).astype(np.uint8)
        prow = np.arange(128, dtype=np.int32).reshape(128, 1)
        if cls == 0:
            prow = prow + 128

        maps.append(
            dict(
                x_idx=x_idx,
                emb=emb,
                wih0T=wih0[dd].astype(bf),
                whh0T=whh0[dd].astype(f8 if USE_FP8 else bf),
                wih1T=wih1[cls][dd].astype(bf),
                whh1T=whh1[dd].astype(f8 if USE_FP8 else bf),
                hwT=hwT[cls],
                fcwT=fcwT[cls],
                aux128=aux128,
                aux17=aux17,
                mren=mren.reshape(1, -1),
                vm=vm,
                msel=msel,
                prow=prow,
            )
        )
    return maps, valid


TRACE = {}


def kernel(**inputs):
    if "nc" not in _CACHE:
        _CACHE["nc"] = _build_nc()
    nc = _CACHE["nc"]
    maps, valid = _prep_maps(inputs)
    kw = {}
    if TRACE.get("on"):
        kw = dict(trace=True, tmpdir=TRACE.get("dir"), trace_cores=[0])
    res = run_bass_kernel_spmd(nc, maps, list(range(NC)), **kw)
    TRACE["last"] = res
    outs = [res.results[i]["out"] for i in range(4)]
    num = np.concatenate([o[0] for o in outs])
    logZ = np.concatenate([o[1] for o in outs])
    aux = np.concatenate([o[2] for o in outs])
    crf_loss = -np.mean(num - logZ, dtype=np.float32)
    aux_loss = np.float32(aux.sum()) / np.float32(max(valid.sum(), 1))
    return np.float32(crf_loss + np.float32(0.1) * aux_loss)


# revision 18
# speedup vs baseline: 1.2276x; 1.1951x over previous
"""BiLSTM-CRF forward loss on 8 TRN2 NeuronCores (Bass/Tile), v3.

v2 "pair plan": each core runs ONE LSTM direction for 8 sequences (direction
encoded in per-core data: backward cores get time-reversed tokens and
k/gate-permuted weights; the SPMD instruction stream is identical). Paired
cores (c, c+4) exchange h histories between layers via pairwise AllGather +
indirect-DMA partner-slot reads. Recurrent weights are fp8 (x32, scale folded
out via activation `scale`), cutting LDWEIGHTS to ~26ns/pair.

v3 on top: gates reordered [g,i,f,o] host-side so the per-step nonlinearity
chain is short and fused (3 adds / 4 activations, single PSUM tile); h is
additionally stored in position-contiguous layouts (histC for own GEMMs, mir
for the partner) so the L1-input / highway GEMMs stream at full rate;
embedding transposes overlap gx0; CRF scan uses batched chain pairs with a
leaner renorm and numerator/aux ops interleaved into its latency gaps.
"""
import sys

import numpy as np

try:
    import concourse  # noqa: F401
except ImportError:  # pragma: no cover
    sys.path.insert(0, "/opt/trn_rl_repo")

import ml_dtypes
from contextlib import ExitStack

import concourse.bass as bass
import concourse.bacc as bacc
import concourse.mybir as mybir
import concourse.tile as tile
from concourse.bass_utils import run_bass_kernel_spmd

F32 = mybir.dt.float32
BF16 = mybir.dt.bfloat16
FP8 = mybir.dt.float8e4
U8 = mybir.dt.uint8
I32 = mybir.dt.int32
AF = mybir.ActivationFunctionType
ALU = mybir.AluOpType
AX = mybir.AxisListType

B, S, E, H, T, V = 32, 256, 256, 512, 17, 50000
NC = 8
NSEQ = 8              # sequences per pair-group (one direction per core)
TOK = NSEQ * S        # 2048 local tokens, flat = 256*q + s
G = 4 * H
GC = G // 128         # 16 gate chunks, device order [g, i, f, o]
KH = H // 128
KE = E // 128
K1 = 2 * H // 128
RENORM = 8
NREN = (S - 1) // RENORM
SPL = 64              # exchange split: slots [SPL:S] shipped at t==192
USE_FP8 = True
WSCALE = 32.0 if USE_FP8 else 1.0
WDT = FP8 if USE_FP8 else BF16

_CACHE = {}


def _build_nc():
    nc = bacc.Bacc(None, target_bir_lowering=False, num_devices=NC)
    d = {}
    P = nc.declare_dram_parameter
    d["x_idx"] = P("x_idx", [128, TOK // 128], I32, isOutput=False)
    d["emb"] = P("emb", [V, E], F32, isOutput=False)
    d["wih0T"] = P("wih0T", [KE, 128, G], BF16, isOutput=False)
    d["whh0T"] = P("whh0T", [KH, 128, G], WDT, isOutput=False)
    d["wih1T"] = P("wih1T", [K1, 128, G], BF16, isOutput=False)
    d["whh1T"] = P("whh1T", [KH, 128, G], WDT, isOutput=False)
    d["hwT"] = P("hwT", [2, K1, 128, 2 * H], BF16, isOutput=False)
    d["fcwT"] = P("fcwT", [128, K1 * T], BF16, isOutput=False)
    d["aux128"] = P("aux128", [128, 48], F32, isOutput=False)
    d["aux17"] = P("aux17", [T, 2220], F32, isOutput=False)
    d["mren"] = P("mren", [1, NREN * NSEQ], F32, isOutput=False)
    d["vm"] = P("vm", [1, TOK], U8, isOutput=False)
    d["msel"] = P("msel", [T, TOK], U8, isOutput=False)
    d["prow"] = P("prow", [128, 1], I32, isOutput=False)
    out_d = P("out", [4, NSEQ], F32, isOutput=True)

    with tile.TileContext(nc) as tc, ExitStack() as ctx:
        pp = ctx.enter_context(tc.tile_pool(name="persist", bufs=1))
        wp = ctx.enter_context(tc.tile_pool(name="wts", bufs=1))
        sp = ctx.enter_context(tc.tile_pool(name="small", bufs=2))
        op = ctx.enter_context(tc.tile_pool(name="once", bufs=1))
        ps = ctx.enter_context(tc.tile_pool(name="psum", bufs=2, space="PSUM"))
        dp = ctx.enter_context(tc.tile_pool(name="dram", bufs=4, space="DRAM"))

        dma = nc.sync.dma_start

        # ---- weights first (prefetch), then small tables -----------------------
        x_sb = pp.tile([128, TOK // 128], I32, tag="xidx")
        dma(x_sb[:], d["x_idx"][:])
        wih_sb = wp.tile([128, K1, G], BF16, tag="wih", name="wih0")
        for k in range(KE):
            dma(wih_sb[:, k, :], d["wih0T"][k])
        whh_sb = wp.tile([128, KH, G], WDT, tag="whh", name="whh0")
        for k in range(KH):
            nc.scalar.dma_start(whh_sb[:, k, :], d["whh0T"][k])
        fcw_sb = pp.tile([128, K1, T], BF16, tag="fcw")
        dma(fcw_sb[:], d["fcwT"][:].rearrange("p (k t) -> p k t", k=K1))
        aux128_sb = pp.tile([128, 48], F32, tag="aux128")
        dma(aux128_sb[:], d["aux128"][:])
        aux17_sb = pp.tile([T, 2220], F32, tag="aux17")
        dma(aux17_sb[:], d["aux17"][:])
        mren_sb = pp.tile([1, NREN, NSEQ], F32, tag="mren")
        dma(mren_sb[:], d["mren"][:].rearrange("o (k b) -> o k b", k=NREN))
        vm_sb = pp.tile([1, NSEQ, S], U8, tag="vm")
        dma(vm_sb[:], d["vm"][:].rearrange("o (b s) -> o b s", b=NSEQ))
        msel_sb = pp.tile([T, NSEQ, S], U8, tag="msel")
        dma(msel_sb[:], d["msel"][:].rearrange("t (b s) -> t b s", b=NSEQ))
        prow_sb = pp.tile([128, 1], I32, tag="prow")
        dma(prow_sb[:], d["prow"][:])

        def b0v(c):
            return aux128_sb[:, c : c + 1]

        def b1v(c):
            return aux128_sb[:, 16 + c : 16 + c + 1]

        def hwbv(w, c):
            return aux128_sb[:, 32 + 8 * w + c : 32 + 8 * w + c + 1]

        trans_sb = aux17_sb[:, 0:T]
        svec_sb = aux17_sb[:, T : T + 1]
        evec_sb = aux17_sb[:, T + 1 : T + 2]
        fcb_sb = aux17_sb[:, T + 2 : T + 3]
        oh_sb = aux17_sb[:, 20 : 20 + TOK].rearrange("t (b s) -> t b s", b=NSEQ)
        cp_base = 20 + TOK
        s0e_sb = aux17_sb[:, cp_base + NSEQ * T : cp_base + NSEQ * T + 2 * NSEQ]

        ones_t = pp.tile([T, 1], F32, tag="onesT")
        nc.vector.memset(ones_t[:], 1.0)
        ones_1t = pp.tile([1, T], F32, tag="ones1T")
        nc.vector.memset(ones_1t[:], 1.0)

        # ---- embedding gather + transpose, pipelined with gx0 ------------------
        XT = pp.tile([128, KE, TOK], BF16, tag="XT")
        gx = pp.tile([128, GC, NSEQ, S], BF16, tag="gx", name="gx0")
        tq = [nc.sync, nc.scalar]
        for b in range(NSEQ):
            for g in range(2):
                embX = sp.tile([128, E], F32, tag="embX")
                nc.gpsimd.indirect_dma_start(
                    out=embX[:],
                    out_offset=None,
                    in_=d["emb"][:],
                    in_offset=bass.IndirectOffsetOnAxis(
                        ap=x_sb[:, 2 * b + g : 2 * b + g + 1], axis=0
                    ),
                )
                embXbf = sp.tile([128, E], BF16, tag="embXb")
                nc.vector.tensor_copy(embXbf[:], embX[:])
                for k in range(KE):
                    tq[(2 * k + g) % 2].dma_start_transpose(
                        XT[:, k, bass.ts(2 * b + g, 128)],
                        embXbf[:, bass.ts(k, 128)],
                    )
            for c in range(GC):
                pt = ps.tile([128, 256], F32, tag="mm")
                for k in range(KE):
                    nc.tensor.matmul(
                        pt[:],
                        wih_sb[:, k, bass.ts(c, 128)],
                        XT[:, k, bass.ts(b, 256)],
                        start=(k == 0),
                        stop=(k == KE - 1),
                    )
                nc.vector.tensor_scalar(
                    out=gx[:, c, b, :], in0=pt[:], scalar1=b0v(c), scalar2=None,
                    op0=ALU.add,
                )

        # ---- recurrence + exchange --------------------------------------------
        RG = [[0, 4], [1, 5], [2, 6], [3, 7]]
        NS = 1.0 / WSCALE

        CHS = 16  # s-positions per exchange chunk (128 KB per contribution)

        def lstm_layer(layer, whh_sb, gxl):
            hist = pp.tile(
                [128, KH, S + 1, NSEQ], BF16, tag="hist", name=f"hist{layer}"
            )
            histC = pp.tile(
                [128, KH, NSEQ, S], BF16, tag="histC", name=f"histC{layer}"
            )
            mir = pp.tile([128, KH, NSEQ, S], BF16, tag="mir", name=f"mir{layer}")
            part = pp.tile(
                [128, KH, NSEQ, S], BF16, tag="part", name=f"part{layer}"
            )

            CWF = KH * NSEQ * CHS

            def ship(i):
                bi = dp.tile([128, CWF], BF16, tag="bi", name=f"bi{layer}_{i}")
                bo = dp.tile([256, CWF], BF16, tag="bo", name=f"bo{layer}_{i}")
                stgo = sp.tile([128, KH * NSEQ, CHS], BF16, tag="stgo", bufs=1)
                nc.vector.tensor_copy(
                    stgo[:], mir[:, :, :, CHS * i : CHS * (i + 1)]
                )
                dma(bi[:], stgo[:].rearrange("p a s -> p (a s)"))
                nc.gpsimd.collective_compute(
                    "AllGather", ALU.bypass, replica_groups=RG,
                    ins=[bi[:].opt()], outs=[bo[:].opt()],
                )
                stgi = sp.tile([128, KH * NSEQ, CHS], BF16, tag="stgi", bufs=1)
                nc.gpsimd.indirect_dma_start(
                    out=stgi[:].rearrange("p a s -> p (a s)"),
                    out_offset=None,
                    in_=bo[:],
                    in_offset=bass.IndirectOffsetOnAxis(ap=prow_sb[:, 0:1], axis=0),
                )
                nc.vector.tensor_copy(
                    part[:, :, :, CHS * i : CHS * (i + 1)], stgi[:]
                )

            nc.vector.memset(hist[:, :, 0, :], 0.0)
            cst = {}
            for par in range(2):
                cst[par] = pp.tile(
                    [128, KH, NSEQ], F32, tag=f"cst{par}", name=f"c{layer}p{par}"
                )
            nc.vector.memset(cst[0][:], 0.0)
            for t in range(S):
                if t >= CHS and t % CHS == 0:
                    ship((S - t) // CHS)
                ptAB = ps.tile([128, GC, NSEQ], F32, tag="rec", bufs=1)
                tmp = sp.tile([128, GC, NSEQ], F32, tag="tmp")
                sig = sp.tile([128, GC, NSEQ], F32, tag="sig")
                for gc in range(GC):
                    for k in range(KH):
                        nc.tensor.matmul(
                            ptAB[:, gc, :],
                            whh_sb[:, k, bass.ts(gc, 128)],
                            hist[:, k, t, :],
                            start=(k == 0),
                            stop=(k == KH - 1),
                        )
                    if gc == 3:
                        # g-gates done: tanh early, overlapping remaining MMs
                        nc.vector.tensor_add(
                            tmp[:, 0:4, :], ptAB[:, 0:4, :], gxl[:, 0:4, :, t]
                        )
                        nc.scalar.activation(
                            sig[:, 0:4, :], tmp[:, 0:4, :], AF.Tanh, scale=NS
                        )
                    elif gc == 11:
                        # i,f gates done: one fused add + sigmoid
                        nc.vector.tensor_add(
                            tmp[:, 4:12, :], ptAB[:, 4:12, :], gxl[:, 4:12, :, t]
                        )
                        nc.scalar.activation(
                            sig[:, 4:12, :], tmp[:, 4:12, :], AF.Sigmoid, scale=NS
                        )
                # o-gates
                nc.vector.tensor_add(
                    tmp[:, 12:16, :], ptAB[:, 12:16, :], gxl[:, 12:16, :, t]
                )
                nc.scalar.activation(
                    sig[:, 12:16, :], tmp[:, 12:16, :], AF.Sigmoid, scale=NS
                )
                c_old, c_new = cst[t % 2], cst[1 - t % 2]
                ig = sp.tile([128, KH, NSEQ], F32, tag="ig")
                nc.vector.tensor_mul(ig[:], sig[:, 4:8, :], sig[:, 0:4, :])
                nc.vector.tensor_mul(c_new[:], sig[:, 8:12, :], c_old[:])
                nc.vector.tensor_add(c_new[:], c_new[:], ig[:])
                th = sp.tile([128, KH, NSEQ], F32, tag="th")
                nc.scalar.activation(th[:], c_new[:], AF.Tanh)
                nc.vector.tensor_mul(hist[:, :, t + 1, :], sig[:, 12:16, :], th[:])
                nc.vector.tensor_copy(histC[:, :, :, t], hist[:, :, t + 1, :])
                nc.vector.tensor_copy(mir[:, :, :, S - 1 - t], hist[:, :, t + 1, :])
            ship(0)
            return histC, part

        histC0, part0 = lstm_layer(0, whh_sb, gx)
        import os
        _KDBG = os.environ.get("KDBG")

        # ---- L1 input GEMM -----------------------------------------------------
        wih_sb = wp.tile([128, K1, G], BF16, tag="wih", name="wih1")
        for k in range(K1):
            dma(wih_sb[:, k, :], d["wih1T"][k])

        def x_slice(histC, part, k, b):
            if k < KH:
                return histC[:, k, b, :]
            return part[:, k - KH, b, :]

        gx1 = pp.tile([128, GC, NSEQ, S], BF16, tag="gx", name="gx1")
        for c in range(GC):
            for b in range(NSEQ):
                pt = ps.tile([128, 256], F32, tag="mm")
                for k in range(K1):
                    nc.tensor.matmul(
                        pt[:],
                        wih_sb[:, k, bass.ts(c, 128)],
                        x_slice(histC0, part0, k, b),
                        start=(k == 0),
                        stop=(k == K1 - 1),
                    )
                nc.vector.tensor_scalar(
                    out=gx1[:, c, b, :], in0=pt[:], scalar1=b1v(c), scalar2=None,
                    op0=ALU.add,
                )

        if _KDBG:
            dbgt = sp.tile([1, NSEQ], F32, tag="dbg")
            nc.vector.tensor_copy(dbgt[:, 0:1], part0[0:1, 1, 0, 100:101])
            nc.vector.tensor_copy(dbgt[:, 1:2], part0[0:1, 0, 0, 10:11])
            nc.vector.tensor_copy(dbgt[:, 2:3], part0[0:1, 0, 0, 63:64])
            nc.vector.tensor_copy(dbgt[:, 3:4], histC0[0:1, 0, 0, 10:11])
            nc.vector.tensor_copy(dbgt[:, 4:5], wih_sb[0:1, 0, 0:1])
            nc.vector.tensor_copy(dbgt[:, 5:6], gx1[0:1, 0, 0, 10:11])
            nc.vector.tensor_copy(dbgt[:, 6:7], gx1[0:1, 0, 0, 100:101])
            nc.vector.tensor_copy(dbgt[:, 7:8], gx1[0:1, 15, 7, 200:201])
            dma(out_d[3:4, :], dbgt[:])

        whh_sb = wp.tile([128, KH, G], WDT, tag="whh", name="whh1")
        for k in range(KH):
            nc.scalar.dma_start(whh_sb[:, k, :], d["whh1T"][k])
        histC1, part1 = lstm_layer(1, whh_sb, gx1)

        # ---- highway + fc ------------------------------------------------------
        hw_sb = pp.tile([128, 2, K1, 2 * H], BF16, tag="gx", name="hw")
        for w in range(2):
            for k in range(K1):
                dma(hw_sb[:, w, k, :], d["hwT"][w, k])
        x2 = wp.tile([128, K1, TOK], BF16, tag="wih", name="x2")
        for c in range(8):
            for b in range(NSEQ):
                ptt = ps.tile([128, 256], F32, tag="mm")
                pth = ps.tile([128, 256], F32, tag="mm")
                for k in range(K1):
                    nc.tensor.matmul(
                        ptt[:], hw_sb[:, 0, k, bass.ts(c, 128)],
                        x_slice(histC1, part1, k, b),
                        start=(k == 0), stop=(k == K1 - 1),
                    )
                for k in range(K1):
                    nc.tensor.matmul(
                        pth[:], hw_sb[:, 1, k, bass.ts(c, 128)],
                        x_slice(histC1, part1, k, b),
                        start=(k == 0), stop=(k == K1 - 1),
                    )
                tg = sp.tile([128, 256], F32, tag="tg")
                nc.scalar.activation(tg[:], ptt[:], AF.Sigmoid, bias=hwbv(0, c))
                rl = sp.tile([128, 256], F32, tag="rl")
                nc.scalar.activation(rl[:], pth[:], AF.Relu, bias=hwbv(1, c))
                dd_ = sp.tile([128, 256], F32, tag="dd")
                nc.vector.tensor_sub(dd_[:], rl[:], x_slice(histC1, part1, c, b))
                nc.vector.tensor_mul(dd_[:], tg[:], dd_[:])
                nc.vector.tensor_add(
                    x2[:, c, bass.ts(b, 256)], dd_[:], x_slice(histC1, part1, c, b)
                )

        logits = pp.tile([T, NSEQ, S], F32, tag="hist", name="logits")
        for b in range(NSEQ):
            pt = ps.tile([128, 256], F32, tag="mm")
            for k in range(K1):
                nc.tensor.matmul(
                    pt[:T, :], fcw_sb[:, k, :], x2[:, k, bass.ts(b, 256)],
                    start=(k == 0), stop=(k == K1 - 1),
                )
            nc.scalar.activation(logits[:, b, :], pt[:T, :], AF.Identity, bias=fcb_sb)

        # ---- CRF + numerator + aux --------------------------------------------
        expEm = pp.tile([T, NSEQ, S], F32, tag="XT", name="expEm")
        nc.scalar.activation(expEm[:], logits[:], AF.Exp)
        expT = op.tile([T, T], F32, tag="expT")
        nc.scalar.activation(expT[:], trans_sb, AF.Exp)
        expS = op.tile([T, 1], F32, tag="expS")
        nc.scalar.activation(expS[:], svec_sb, AF.Exp)
        expE = op.tile([T, 1], F32, tag="expE")
        nc.scalar.activation(expE[:], evec_sb, AF.Exp)

        # numerator / aux emitters, interleaved into CRF latency gaps
        emm = pp.tile([T, NSEQ, S], F32, tag="mir", name="emm")
        empart = sp.tile([T, NSEQ], F32, tag="empart")
        nv = sp.tile([T, NSEQ], F32, tag="nv")
        ev = sp.tile([T, NSEQ], F32, tag="ev")
        lse = pp.tile([1, NSEQ, S], F32, tag="part", name="lse")
        num_sb = sp.tile([1, NSEQ], F32, tag="num")
        aux_sb = sp.tile([1, NSEQ], F32, tag="aux")
        trp = op.tile([T, T], F32, tag="trp")
        trr = sp.tile([T, NSEQ], F32, tag="trr")

        fills = []

        def emit_fills():
            fills.append(lambda: nc.vector.tensor_mul(emm[:], logits[:], oh_sb))
            fills.append(
                lambda: nc.vector.reduce_sum(empart[:], emm[:], axis=AX.X)
            )
            fills.append(
                lambda: nc.vector.tensor_scalar(
                    out=nv[:], in0=s0e_sb[:, 0:NSEQ], scalar1=svec_sb,
                    scalar2=None, op0=ALU.mult,
                )
            )
            fills.append(
                lambda: nc.vector.tensor_scalar(
                    out=ev[:], in0=s0e_sb[:, NSEQ : 2 * NSEQ], scalar1=evec_sb,
                    scalar2=None, op0=ALU.mult,
                )
            )
            fills.append(lambda: nc.vector.tensor_add(nv[:], nv[:], ev[:]))
            fills.append(lambda: nc.vector.tensor_add(nv[:], nv[:], empart[:]))
            for bb in range(NSEQ):
                fills.append(
                    lambda bb=bb: nc.vector.tensor_mul(
                        trp[:],
                        aux17_sb[:, cp_base + T * bb : cp_base + T * (bb + 1)],
                        trans_sb,
                    )
                )
                fills.append(
                    lambda bb=bb: nc.vector.reduce_sum(
                        trr[:, bb : bb + 1], trp[:], axis=AX.X
                    )
                )
            for hlf in range(4):
                def aux_lse(hlf=hlf):
                    psE = ps.tile([1, 512], F32, tag="small")
                    nc.tensor.matmul(
                        psE[:, :512], ones_t[:],
                        expEm[:, 2 * hlf : 2 * hlf + 2, :],
                        start=True, stop=True,
                    )
                    nc.scalar.activation(
                        lse[:, 2 * hlf : 2 * hlf + 2, :],
                        psE[:].rearrange("o (b s) -> o b s", b=2),
                        AF.Ln,
                    )
                fills.append(aux_lse)

        emit_fills()

        afin = pp.tile([T, NSEQ], F32, tag="afin")
        lacc = {}
        for ch in range(2):
            for par in range(2):
                lacc[(ch, par)] = pp.tile(
                    [1, 4], F32, tag=f"lacc{ch}{par}", name=f"lacc{ch}{par}"
                )
            nc.vector.memset(lacc[(ch, 0)][:], 0.0)
        ap = ctx.enter_context(tc.tile_pool(name="crf", bufs=4))

        A = {}
        for ch in range(2):
            sl = slice(4 * ch, 4 * ch + 4)
            A[ch] = ap.tile([T, 4], F32, tag=f"A{ch}", name=f"A{ch}")
            nc.vector.tensor_scalar(
                out=A[ch][:], in0=expEm[:, sl, 0], scalar1=expS[:, 0:1],
                scalar2=None, op0=ALU.mult,
            )
        nren_seen = 0
        for t in range(1, S):
            for ch in range(2):
                sl = slice(4 * ch, 4 * ch + 4)
                pt = ps.tile([128, 4], F32, tag="mm")
                nc.tensor.matmul(pt[:T, :], expT[:], A[ch][:], start=True, stop=True)
                A[ch] = ap.tile([T, 4], F32, tag=f"A{ch}", name=f"A{ch}")
                nc.vector.tensor_mul(A[ch][:], pt[:T, :], expEm[:, sl, t])
            if fills:
                fills.pop(0)()
            if t % RENORM == 0:
                for ch in range(2):
                    sl = slice(4 * ch, 4 * ch + 4)
                    psS = ps.tile([1, 512], F32, tag="small")
                    nc.tensor.matmul(
                        psS[:, :4], ones_t[:], A[ch][:], start=True, stop=True
                    )
                    Sr = ap.tile([1, 4], F32, tag=f"Sr{ch}", name=f"Sr{ch}")
                    nc.vector.reciprocal(Sr[:], psS[:, :4])
                    lnS = ap.tile([1, 4], F32, tag=f"lnS{ch}", name=f"lnS{ch}")
                    nc.scalar.activation(lnS[:], psS[:, :4], AF.Ln)
                    pB = ps.tile([128, 4], F32, tag="mm")
                    nc.tensor.matmul(pB[:T, :], ones_1t[:], Sr[:], start=True, stop=True)
                    A2 = ap.tile([T, 4], F32, tag=f"A{ch}", name=f"A{ch}")
                    nc.vector.tensor_mul(A2[:], A[ch][:], pB[:T, :])
                    A[ch] = A2
                    nc.vector.tensor_mul(lnS[:], lnS[:], mren_sb[:, nren_seen, sl])
                    old, new = lacc[(ch, nren_seen % 2)], lacc[(ch, 1 - nren_seen % 2)]
                    nc.vector.tensor_add(new[:], old[:], lnS[:])
                nren_seen += 1
            if t >= S // 2 - 1:
                for ch in range(2):
                    sl = slice(4 * ch, 4 * ch + 4)
                    nc.vector.copy_predicated(
                        afin[:, sl], msel_sb[:, sl, t], A[ch][:]
                    )
        for f in fills:
            f()

        ae = op.tile([T, NSEQ], F32, tag="ae")
        nc.vector.tensor_scalar(
            out=ae[:], in0=afin[:], scalar1=expE[:, 0:1], scalar2=None, op0=ALU.mult
        )
        psZ = ps.tile([1, 512], F32, tag="small")
        nc.tensor.matmul(psZ[:, :NSEQ], ones_t[:], ae[:], start=True, stop=True)
        logZ = sp.tile([1, NSEQ], F32, tag="logZ")
        nc.scalar.activation(logZ[:], psZ[:, :NSEQ], AF.Ln)
        for ch in range(2):
            sl = slice(4 * ch, 4 * ch + 4)
            nc.vector.tensor_add(
                logZ[:, sl], logZ[:, sl], lacc[(ch, nren_seen % 2)][:]
            )

        # numerator wrap-up (trr columns were filled per-seq during the scan)
        nc.vector.tensor_add(nv[:], nv[:], trr[:])
        psN = ps.tile([1, 512], F32, tag="small")
        nc.tensor.matmul(psN[:, :NSEQ], ones_t[:], nv[:], start=True, stop=True)
        nc.vector.tensor_copy(num_sb[:], psN[:, :NSEQ])

        nc.vector.tensor_mul(lse[:], lse[:], vm_sb)
        lsum = sp.tile([1, NSEQ], F32, tag="lsum")
        nc.vector.reduce_sum(lsum[:], lse[:], axis=AX.X)
        psM = ps.tile([1, 512], F32, tag="small")
        nc.tensor.matmul(psM[:, :NSEQ], ones_t[:], empart[:], start=True, stop=True)
        nc.vector.tensor_sub(aux_sb[:], lsum[:], psM[:, :NSEQ])

        dma(out_d[0:1, :], num_sb[:])
        dma(out_d[1:2, :], logZ[:])
        dma(out_d[2:3, :], aux_sb[:])

    nc.compile()
    return nc


PG = np.concatenate(
    [np.arange(2 * H, 3 * H), np.arange(0, H), np.arange(H, 2 * H),
     np.arange(3 * H, 4 * H)]
)  # PyTorch gate rows [i,f,g,o] -> device order [g,i,f,o]


def _prep_maps(inputs):
    bf = ml_dtypes.bfloat16
    f8 = ml_dtypes.float8_e4m3
    x = np.asarray(inputs["x"]).astype(np.int32)
    tags = np.asarray(inputs["tags"]).astype(np.int32)
    emb = np.asarray(inputs["emb"], np.float32)
    sc = np.float32(WSCALE)

    def t_chunks(w, perm=None):
        # (G, K) -> (K//128, 128, G) with gate-row reorder; optional input perm
        w = np.asarray(w, np.float32)[PG]
        K = w.shape[1]
        wT = np.ascontiguousarray(w.T).reshape(K // 128, 128, G)
        if perm is not None:
            wT = wT[perm]
        return wT

    wih0 = [t_chunks(sc * np.asarray(inputs["w_ih_l0"], np.float32)[dd]) for dd in range(2)]
    whh0 = [t_chunks(sc * np.asarray(inputs["w_hh_l0"], np.float32)[dd]) for dd in range(2)]
    whh1 = [t_chunks(sc * np.asarray(inputs["w_hh_l1"], np.float32)[dd]) for dd in range(2)]
    swap = [4, 5, 6, 7, 0, 1, 2, 3]
    wih1 = {}
    for cls in range(2):
        perm = None if cls == 0 else swap
        wih1[cls] = [
            t_chunks(sc * np.asarray(inputs["w_ih_l1"], np.float32)[dd], perm)
            for dd in range(2)
        ]

    PH = np.arange(2 * H)
    PH_swap = np.concatenate([PH[H:], PH[:H]])
    hw_t = np.asarray(inputs["hw_t_w"], np.float32)
    hw_h = np.asarray(inputs["hw_h_w"], np.float32)
    hw_tb = np.asarray(inputs["hw_t_b"], np.float32)
    hw_hb = np.asarray(inputs["hw_h_b"], np.float32)
    fcw = np.asarray(inputs["fc_w"], np.float32)
    hwT, fcwT, hwb = {}, {}, {}
    for cls in range(2):
        pr = PH if cls == 0 else PH_swap
        ht = hw_t[np.ix_(pr, pr)]
        hh = hw_h[np.ix_(pr, pr)]
        hwT[cls] = np.stack(
            [ht.T.reshape(K1, 128, 2 * H), hh.T.reshape(K1, 128, 2 * H)]
        ).astype(bf)
        fcwT[cls] = (
            np.ascontiguousarray(fcw[:, pr].T)
            .reshape(K1, 128, T)
            .transpose(1, 0, 2)
            .reshape(128, K1 * T)
            .astype(bf)
        )
        hwb[cls] = (
            hw_tb[pr].reshape(8, 128).T,
            hw_hb[pr].reshape(8, 128).T,
        )

    b0 = (sc * np.asarray(inputs["b_l0"], np.float32))[:, PG].reshape(2, GC, 128)
    b1 = (sc * np.asarray(inputs["b_l1"], np.float32))[:, PG].reshape(2, GC, 128)
    trans = np.asarray(inputs["crf_trans"], np.float32)
    svec = np.asarray(inputs["crf_start"], np.float32)
    evec = np.asarray(inputs["crf_end"], np.float32)
    fcb = np.asarray(inputs["fc_b"], np.float32)

    valid = tags != 0
    lengths = (x != 0).sum(1)

    maps = []
    for core in range(NC):
        cls = 0 if core < 4 else 1
        dd = cls
        g = core % 4
        sl = slice(g * NSEQ, (g + 1) * NSEQ)
        xl_nat, tl, vl, ll = x[sl], tags[sl], valid[sl], lengths[sl]
        xl = xl_nat if cls == 0 else xl_nat[:, ::-1]
        flat = np.ascontiguousarray(xl).reshape(-1)
        x_idx = np.ascontiguousarray(flat.reshape(TOK // 128, 128).T).astype(np.int32)

        aux128 = np.zeros((128, 48), np.float32)
        aux128[:, 0:16] = b0[dd].reshape(16, 128).T
        aux128[:, 16:32] = b1[dd].reshape(16, 128).T
        aux128[:, 32:40] = hwb[cls][0]
        aux128[:, 40:48] = hwb[cls][1]

        jj = np.arange(T)
        oh = (tl[None, :, :] == jj[:, None, None]) & vl[None, :, :]
        oh_tags = oh.reshape(T, TOK).astype(np.float32)
        cp = np.zeros((T, NSEQ, T), np.float32)
        prev, cur = tl[:, :-1], tl[:, 1:]
        vstep = vl[:, 1:]
        for b in range(NSEQ):
            np.add.at(cp[:, b, :], (prev[b][vstep[b]], cur[b][vstep[b]]), 1.0)
        s0e = np.zeros((T, 2 * NSEQ), np.float32)
        for b in range(NSEQ):
            s0e[tl[b, 0], b] = 1.0
            s0e[tl[b, ll[b] - 1], NSEQ + b] = 1.0
        msel = np.zeros((NSEQ, S), np.float32)
        for b in range(NSEQ):
            msel[b, ll[b] - 1] = 1.0
        msel = np.broadcast_to(msel.reshape(1, TOK), (T, TOK)).astype(np.uint8)
        mren = np.zeros((NREN, NSEQ), np.float32)
        for k in range(NREN):
            mren[k] = (RENORM * (k + 1) <= ll - 1).astype(np.float32)
        aux17 = np.zeros((T, 2220), np.float32)
        aux17[:, 0:T] = trans
        aux17[:, T] = svec
        aux17[:, T + 1] = evec
        aux17[:, T + 2] = fcb
        aux17[:, 20 : 20 + TOK] = oh_tags
        aux17[:, 20 + TOK : 20 + TOK + NSEQ * T] = cp.reshape(T, NSEQ * T)
        aux17[:, 20 + TOK + NSEQ * T : 20 + TOK + NSEQ * T + 2 * NSEQ] = s0e
        vm = vl.reshape(1, TOK).astype(np.uint8)
        prow = np.arange(128, dtype=np.int32).reshape(128, 1)
        if cls == 0:
            prow = prow + 128

        maps.append(
            dict(
                x_idx=x_idx,
                emb=emb,
                wih0T=wih0[dd].astype(bf),
                whh0T=whh0[dd].astype(f8 if USE_FP8 else bf),
                wih1T=wih1[cls][dd].astype(bf),
                whh1T=whh1[dd].astype(f8 if USE_FP8 else bf),
                hwT=hwT[cls],
                fcwT=fcwT[cls],
                aux128=aux128,
                aux17=aux17,
                mren=mren.reshape(1, -1),
                vm=vm,
                msel=msel,
                prow=prow,
            )
        )
    return maps, valid


TRACE = {}


def kernel(**inputs):
    if "nc" not in _CACHE:
        _CACHE["nc"] = _build_nc()
    nc = _CACHE["nc"]
    maps, valid = _prep_maps(inputs)
    kw = {}
    if TRACE.get("on"):
        kw = dict(trace=True, tmpdir=TRACE.get("dir"), trace_cores=[0])
    res = run_bass_kernel_spmd(nc, maps, list(range(NC)), **kw)
    TRACE["last"] = res
    outs = [res.results[i]["out"] for i in range(4)]
    num = np.concatenate([o[0] for o in outs])
    logZ = np.concatenate([o[1] for o in outs])
    aux = np.concatenate([o[2] for o in outs])
    crf_loss = -np.mean(num - logZ, dtype=np.float32)
    aux_loss = np.float32(aux.sum()) / np.float32(max(valid.sum(), 1))
    return np.float32(crf_loss + np.float32(0.1) * aux_loss)


# revision 19
# speedup vs baseline: 1.4544x; 1.1848x over previous
"""BiLSTM-CRF forward loss on 8 TRN2 NeuronCores (Bass/Tile), v3.

v2 "pair plan": each core runs ONE LSTM direction for 8 sequences (direction
encoded in per-core data: backward cores get time-reversed tokens and
k/gate-permuted weights; the SPMD instruction stream is identical). Paired
cores (c, c+4) exchange h histories between layers via pairwise AllGather +
indirect-DMA partner-slot reads. Recurrent weights are fp8 (x32, scale folded
out via activation `scale`), cutting LDWEIGHTS to ~26ns/pair.

v3 on top: gates reordered [g,i,f,o] host-side so the per-step nonlinearity
chain is short and fused (3 adds / 4 activations, single PSUM tile); h is
additionally stored in position-contiguous layouts (histC for own GEMMs, mir
for the partner) so the L1-input / highway GEMMs stream at full rate;
embedding transposes overlap gx0; CRF scan uses batched chain pairs with a
leaner renorm and numerator/aux ops interleaved into its latency gaps.
"""
import sys

import numpy as np

try:
    import concourse  # noqa: F401
except ImportError:  # pragma: no cover
    sys.path.insert(0, "/opt/trn_rl_repo")

import ml_dtypes
from contextlib import ExitStack

import concourse.bass as bass
import concourse.bacc as bacc
import concourse.mybir as mybir
import concourse.tile as tile
from concourse.bass_utils import run_bass_kernel_spmd

F32 = mybir.dt.float32
BF16 = mybir.dt.bfloat16
FP8 = mybir.dt.float8e4
U8 = mybir.dt.uint8
I32 = mybir.dt.int32
AF = mybir.ActivationFunctionType
ALU = mybir.AluOpType
AX = mybir.AxisListType

B, S, E, H, T, V = 32, 256, 256, 512, 17, 50000
NC = 8
NSEQ = 8              # sequences per pair-group (one direction per core)
TOK = NSEQ * S        # 2048 local tokens, flat = 256*q + s
G = 4 * H
GC = G // 128         # 16 gate chunks, device order [g, i, f, o]
KH = H // 128
KE = E // 128
K1 = 2 * H // 128
RENORM = 8
NREN = (S - 1) // RENORM
SPL = 64              # exchange split: slots [SPL:S] shipped at t==192
USE_FP8 = True
WSCALE = 32.0 if USE_FP8 else 1.0
WDT = FP8 if USE_FP8 else BF16

_CACHE = {}


def _build_nc():
    nc = bacc.Bacc(None, target_bir_lowering=False, num_devices=NC)
    d = {}
    P = nc.declare_dram_parameter
    d["x_idx"] = P("x_idx", [128, TOK // 128], I32, isOutput=False)
    d["emb"] = P("emb", [V, E], F32, isOutput=False)
    d["wih0T"] = P("wih0T", [KE, 128, G], BF16, isOutput=False)
    d["whh0T"] = P("whh0T", [KH, 128, G], WDT, isOutput=False)
    d["wih1T"] = P("wih1T", [K1, 128, G], BF16, isOutput=False)
    d["whh1T"] = P("whh1T", [KH, 128, G], WDT, isOutput=False)
    d["hwT"] = P("hwT", [2, K1, 128, 2 * H], BF16, isOutput=False)
    d["fcwT"] = P("fcwT", [128, K1 * T], BF16, isOutput=False)
    d["aux128"] = P("aux128", [128, 48], F32, isOutput=False)
    d["aux17"] = P("aux17", [T, 2220], F32, isOutput=False)
    d["mren"] = P("mren", [1, NREN * NSEQ], F32, isOutput=False)
    d["vm"] = P("vm", [1, TOK], U8, isOutput=False)
    d["msel"] = P("msel", [T, TOK], U8, isOutput=False)
    d["prow"] = P("prow", [128, 1], I32, isOutput=False)
    out_d = P("out", [4, NSEQ], F32, isOutput=True)

    with tile.TileContext(nc) as tc, ExitStack() as ctx:
        pp = ctx.enter_context(tc.tile_pool(name="persist", bufs=1))
        wp = ctx.enter_context(tc.tile_pool(name="wts", bufs=1))
        sp = ctx.enter_context(tc.tile_pool(name="small", bufs=2))
        op = ctx.enter_context(tc.tile_pool(name="once", bufs=1))
        ps = ctx.enter_context(tc.tile_pool(name="psum", bufs=2, space="PSUM"))
        dp = ctx.enter_context(tc.tile_pool(name="dram", bufs=4, space="DRAM"))

        dma = nc.sync.dma_start

        # ---- weights first (prefetch), then small tables -----------------------
        x_sb = pp.tile([128, TOK // 128], I32, tag="xidx")
        dma(x_sb[:], d["x_idx"][:])
        wih_sb = wp.tile([128, K1, G], BF16, tag="wih", name="wih0")
        for k in range(KE):
            dma(wih_sb[:, k, :], d["wih0T"][k])
        whh_sb = wp.tile([128, KH, G], WDT, tag="whh", name="whh0")
        for k in range(KH):
            nc.scalar.dma_start(whh_sb[:, k, :], d["whh0T"][k])
        fcw_sb = pp.tile([128, K1, T], BF16, tag="fcw")
        dma(fcw_sb[:], d["fcwT"][:].rearrange("p (k t) -> p k t", k=K1))
        aux128_sb = pp.tile([128, 48], F32, tag="aux128")
        dma(aux128_sb[:], d["aux128"][:])
        aux17_sb = pp.tile([T, 2220], F32, tag="aux17")
        dma(aux17_sb[:], d["aux17"][:])
        mren_sb = pp.tile([1, NREN, NSEQ], F32, tag="mren")
        dma(mren_sb[:], d["mren"][:].rearrange("o (k b) -> o k b", k=NREN))
        vm_sb = pp.tile([1, NSEQ, S], U8, tag="vm")
        dma(vm_sb[:], d["vm"][:].rearrange("o (b s) -> o b s", b=NSEQ))
        msel_sb = pp.tile([T, NSEQ, S], U8, tag="msel")
        dma(msel_sb[:], d["msel"][:].rearrange("t (b s) -> t b s", b=NSEQ))
        prow_sb = pp.tile([128, 1], I32, tag="prow")
        dma(prow_sb[:], d["prow"][:])

        def b0v(c):
            return aux128_sb[:, c : c + 1]

        def b1v(c):
            return aux128_sb[:, 16 + c : 16 + c + 1]

        def hwbv(w, c):
            return aux128_sb[:, 32 + 8 * w + c : 32 + 8 * w + c + 1]

        trans_sb = aux17_sb[:, 0:T]
        svec_sb = aux17_sb[:, T : T + 1]
        evec_sb = aux17_sb[:, T + 1 : T + 2]
        fcb_sb = aux17_sb[:, T + 2 : T + 3]
        oh_sb = aux17_sb[:, 20 : 20 + TOK].rearrange("t (b s) -> t b s", b=NSEQ)
        cp_base = 20 + TOK
        s0e_sb = aux17_sb[:, cp_base + NSEQ * T : cp_base + NSEQ * T + 2 * NSEQ]

        ones_t = pp.tile([T, 1], F32, tag="onesT")
        nc.vector.memset(ones_t[:], 1.0)
        ones_1t = pp.tile([1, T], F32, tag="ones1T")
        nc.vector.memset(ones_1t[:], 1.0)

        # ---- embedding gather + transpose, pipelined with gx0 ------------------
        XT = pp.tile([128, KE, TOK], BF16, tag="XT")
        gx = pp.tile([128, GC, NSEQ, S], BF16, tag="gx", name="gx0")
        tq = [nc.sync, nc.scalar]
        for g in range(2 * NSEQ):
            embX = sp.tile([128, E], F32, tag="embX")
            nc.gpsimd.indirect_dma_start(
                out=embX[:],
                out_offset=None,
                in_=d["emb"][:],
                in_offset=bass.IndirectOffsetOnAxis(
                    ap=x_sb[:, g : g + 1], axis=0
                ),
            )
            embXbf = sp.tile([128, E], BF16, tag="embXb")
            nc.vector.tensor_copy(embXbf[:], embX[:])
            for k in range(KE):
                tq[(2 * k + g) % 2].dma_start_transpose(
                    XT[:, k, bass.ts(g, 128)],
                    embXbf[:, bass.ts(k, 128)],
                )
        for b in range(NSEQ):
            for c in range(GC):
                pt = ps.tile([128, 256], F32, tag="mm")
                for k in range(KE):
                    nc.tensor.matmul(
                        pt[:],
                        wih_sb[:, k, bass.ts(c, 128)],
                        XT[:, k, bass.ts(b, 256)],
                        start=(k == 0),
                        stop=(k == KE - 1),
                    )
                nc.vector.tensor_scalar(
                    out=gx[:, c, b, :], in0=pt[:], scalar1=b0v(c), scalar2=None,
                    op0=ALU.add,
                )

        # ---- recurrence + exchange --------------------------------------------
        RG = [[0, 4], [1, 5], [2, 6], [3, 7]]
        NS = 1.0 / WSCALE

        CHS = 16  # s-positions per exchange chunk (128 KB per contribution)

        def lstm_layer(layer, whh_sb, gxl):
            hist = pp.tile(
                [128, KH, S + 1, NSEQ], BF16, tag="hist", name=f"hist{layer}"
            )
            histC = pp.tile(
                [128, KH, NSEQ, S], BF16, tag="histC", name=f"histC{layer}"
            )
            mir = pp.tile([128, KH, NSEQ, S], BF16, tag="mir", name=f"mir{layer}")
            part = pp.tile(
                [128, KH, NSEQ, S], BF16, tag="part", name=f"part{layer}"
            )

            CWF = KH * NSEQ * CHS

            def ship(i):
                bi = dp.tile([128, CWF], BF16, tag="bi", name=f"bi{layer}_{i}")
                bo = dp.tile([256, CWF], BF16, tag="bo", name=f"bo{layer}_{i}")
                stgo = sp.tile([128, KH * NSEQ, CHS], BF16, tag="stgo", bufs=1)
                nc.vector.tensor_copy(
                    stgo[:], mir[:, :, :, CHS * i : CHS * (i + 1)]
                )
                dma(bi[:], stgo[:].rearrange("p a s -> p (a s)"))
                nc.gpsimd.collective_compute(
                    "AllGather", ALU.bypass, replica_groups=RG,
                    ins=[bi[:].opt()], outs=[bo[:].opt()],
                )
                stgi = sp.tile([128, KH * NSEQ, CHS], BF16, tag="stgi", bufs=1)
                nc.gpsimd.indirect_dma_start(
                    out=stgi[:].rearrange("p a s -> p (a s)"),
                    out_offset=None,
                    in_=bo[:],
                    in_offset=bass.IndirectOffsetOnAxis(ap=prow_sb[:, 0:1], axis=0),
                )
                nc.vector.tensor_copy(
                    part[:, :, :, CHS * i : CHS * (i + 1)], stgi[:]
                )

            nc.vector.memset(hist[:, :, 0, :], 0.0)
            cst = {}
            for par in range(2):
                cst[par] = pp.tile(
                    [128, KH, NSEQ], F32, tag=f"cst{par}", name=f"c{layer}p{par}"
                )
            nc.vector.memset(cst[0][:], 0.0)
            for t in range(S):
                if t >= CHS and t % CHS == 0:
                    ship((S - t) // CHS)
                ptG = ps.tile([128, 4, NSEQ], F32, tag="recG", bufs=1)
                ptIF = ps.tile([128, 8, NSEQ], F32, tag="recIF", bufs=1)
                ptO = ps.tile([128, 4, NSEQ], F32, tag="recO", bufs=1)
                tmp = sp.tile([128, GC, NSEQ], F32, tag="tmp")
                sig = sp.tile([128, GC, NSEQ], F32, tag="sig")
                for gc in range(GC):
                    dst = (
                        ptG[:, gc, :] if gc < 4
                        else (ptIF[:, gc - 4, :] if gc < 12 else ptO[:, gc - 12, :])
                    )
                    for k in range(KH):
                        nc.tensor.matmul(
                            dst,
                            whh_sb[:, k, bass.ts(gc, 128)],
                            hist[:, k, t, :],
                            start=(k == 0),
                            stop=(k == KH - 1),
                        )
                    if gc == 3:
                        # g-gates done: tanh early, overlapping remaining MMs
                        nc.vector.tensor_add(
                            tmp[:, 0:4, :], ptG[:], gxl[:, 0:4, :, t]
                        )
                        nc.scalar.activation(
                            sig[:, 0:4, :], tmp[:, 0:4, :], AF.Tanh, scale=NS
                        )
                    elif gc == 11:
                        # i,f gates done: one fused add + sigmoid
                        nc.vector.tensor_add(
                            tmp[:, 4:12, :], ptIF[:], gxl[:, 4:12, :, t]
                        )
                        nc.scalar.activation(
                            sig[:, 4:12, :], tmp[:, 4:12, :], AF.Sigmoid, scale=NS
                        )
                # o-gates
                nc.vector.tensor_add(
                    tmp[:, 12:16, :], ptO[:], gxl[:, 12:16, :, t]
                )
                nc.scalar.activation(
                    sig[:, 12:16, :], tmp[:, 12:16, :], AF.Sigmoid, scale=NS
                )
                c_old, c_new = cst[t % 2], cst[1 - t % 2]
                ig = sp.tile([128, KH, NSEQ], F32, tag="ig")
                nc.vector.tensor_mul(ig[:], sig[:, 4:8, :], sig[:, 0:4, :])
                nc.vector.tensor_mul(c_new[:], sig[:, 8:12, :], c_old[:])
                nc.vector.tensor_add(c_new[:], c_new[:], ig[:])
                th = sp.tile([128, KH, NSEQ], F32, tag="th")
                nc.scalar.activation(th[:], c_new[:], AF.Tanh)
                nc.vector.tensor_mul(hist[:, :, t + 1, :], sig[:, 12:16, :], th[:])
                nc.scalar.activation(
                    histC[:, :, :, t], hist[:, :, t + 1, :], AF.Identity
                )
                nc.gpsimd.tensor_copy(mir[:, :, :, S - 1 - t], hist[:, :, t + 1, :])
            ship(0)
            return histC, part

        histC0, part0 = lstm_layer(0, whh_sb, gx)
        import os
        _KDBG = os.environ.get("KDBG")

        # ---- L1 input GEMM -----------------------------------------------------
        wih_sb = wp.tile([128, K1, G], BF16, tag="wih", name="wih1")
        for k in range(K1):
            dma(wih_sb[:, k, :], d["wih1T"][k])

        def x_slice(histC, part, k, b):
            if k < KH:
                return histC[:, k, b, :]
            return part[:, k - KH, b, :]

        gx1 = pp.tile([128, GC, NSEQ, S], BF16, tag="gx", name="gx1")
        for c in range(GC):
            for b in range(NSEQ):
                pt = ps.tile([128, 256], F32, tag="mm")
                for k in range(K1):
                    nc.tensor.matmul(
                        pt[:],
                        wih_sb[:, k, bass.ts(c, 128)],
                        x_slice(histC0, part0, k, b),
                        start=(k == 0),
                        stop=(k == K1 - 1),
                    )
                nc.vector.tensor_scalar(
                    out=gx1[:, c, b, :], in0=pt[:], scalar1=b1v(c), scalar2=None,
                    op0=ALU.add,
                )

        if _KDBG:
            dbgt = sp.tile([1, NSEQ], F32, tag="dbg")
            nc.vector.tensor_copy(dbgt[:, 0:1], part0[0:1, 1, 0, 100:101])
            nc.vector.tensor_copy(dbgt[:, 1:2], part0[0:1, 0, 0, 10:11])
            nc.vector.tensor_copy(dbgt[:, 2:3], part0[0:1, 0, 0, 63:64])
            nc.vector.tensor_copy(dbgt[:, 3:4], histC0[0:1, 0, 0, 10:11])
            nc.vector.tensor_copy(dbgt[:, 4:5], wih_sb[0:1, 0, 0:1])
            nc.vector.tensor_copy(dbgt[:, 5:6], gx1[0:1, 0, 0, 10:11])
            nc.vector.tensor_copy(dbgt[:, 6:7], gx1[0:1, 0, 0, 100:101])
            nc.vector.tensor_copy(dbgt[:, 7:8], gx1[0:1, 15, 7, 200:201])
            dma(out_d[3:4, :], dbgt[:])

        whh_sb = wp.tile([128, KH, G], WDT, tag="whh", name="whh1")
        for k in range(KH):
            nc.scalar.dma_start(whh_sb[:, k, :], d["whh1T"][k])
        histC1, part1 = lstm_layer(1, whh_sb, gx1)

        # ---- highway + fc ------------------------------------------------------
        hw_sb = pp.tile([128, 2, K1, 2 * H], BF16, tag="gx", name="hw")
        for w in range(2):
            for k in range(K1):
                dma(hw_sb[:, w, k, :], d["hwT"][w, k])
        x2 = wp.tile([128, K1, TOK], BF16, tag="wih", name="x2")
        for c in range(8):
            for b in range(NSEQ):
                ptt = ps.tile([128, 256], F32, tag="mm")
                pth = ps.tile([128, 256], F32, tag="mm")
                for k in range(K1):
                    nc.tensor.matmul(
                        ptt[:], hw_sb[:, 0, k, bass.ts(c, 128)],
                        x_slice(histC1, part1, k, b),
                        start=(k == 0), stop=(k == K1 - 1),
                    )
                for k in range(K1):
                    nc.tensor.matmul(
                        pth[:], hw_sb[:, 1, k, bass.ts(c, 128)],
                        x_slice(histC1, part1, k, b),
                        start=(k == 0), stop=(k == K1 - 1),
                    )
                tg = sp.tile([128, 256], F32, tag="tg")
                nc.scalar.activation(tg[:], ptt[:], AF.Sigmoid, bias=hwbv(0, c))
                rl = sp.tile([128, 256], F32, tag="rl")
                nc.scalar.activation(rl[:], pth[:], AF.Relu, bias=hwbv(1, c))
                dd_ = sp.tile([128, 256], F32, tag="dd")
                nc.vector.tensor_sub(dd_[:], rl[:], x_slice(histC1, part1, c, b))
                nc.vector.tensor_mul(dd_[:], tg[:], dd_[:])
                nc.vector.tensor_add(
                    x2[:, c, bass.ts(b, 256)], dd_[:], x_slice(histC1, part1, c, b)
                )

        logits = pp.tile([T, NSEQ, S], F32, tag="hist", name="logits")
        for b in range(NSEQ):
            pt = ps.tile([128, 256], F32, tag="mm")
            for k in range(K1):
                nc.tensor.matmul(
                    pt[:T, :], fcw_sb[:, k, :], x2[:, k, bass.ts(b, 256)],
                    start=(k == 0), stop=(k == K1 - 1),
                )
            nc.scalar.activation(logits[:, b, :], pt[:T, :], AF.Identity, bias=fcb_sb)

        # ---- CRF + numerator + aux --------------------------------------------
        expEm = pp.tile([T, NSEQ, S], F32, tag="XT", name="expEm")
        nc.scalar.activation(expEm[:], logits[:], AF.Exp)
        expT = op.tile([T, T], F32, tag="expT")
        nc.scalar.activation(expT[:], trans_sb, AF.Exp)
        expS = op.tile([T, 1], F32, tag="expS")
        nc.scalar.activation(expS[:], svec_sb, AF.Exp)
        expE = op.tile([T, 1], F32, tag="expE")
        nc.scalar.activation(expE[:], evec_sb, AF.Exp)

        # numerator / aux emitters, interleaved into CRF latency gaps
        emm = pp.tile([T, NSEQ, S], F32, tag="mir", name="emm")
        empart = sp.tile([T, NSEQ], F32, tag="empart")
        nv = sp.tile([T, NSEQ], F32, tag="nv")
        ev = sp.tile([T, NSEQ], F32, tag="ev")
        lse = pp.tile([1, NSEQ, S], F32, tag="part", name="lse")
        num_sb = sp.tile([1, NSEQ], F32, tag="num")
        aux_sb = sp.tile([1, NSEQ], F32, tag="aux")
        trp = op.tile([T, T], F32, tag="trp")
        trr = sp.tile([T, NSEQ], F32, tag="trr")

        fills = []

        def emit_fills():
            fills.append(lambda: nc.vector.tensor_mul(emm[:], logits[:], oh_sb))
            fills.append(
                lambda: nc.vector.reduce_sum(empart[:], emm[:], axis=AX.X)
            )
            fills.append(
                lambda: nc.vector.tensor_scalar(
                    out=nv[:], in0=s0e_sb[:, 0:NSEQ], scalar1=svec_sb,
                    scalar2=None, op0=ALU.mult,
                )
            )
            fills.append(
                lambda: nc.vector.tensor_scalar(
                    out=ev[:], in0=s0e_sb[:, NSEQ : 2 * NSEQ], scalar1=evec_sb,
                    scalar2=None, op0=ALU.mult,
                )
            )
            fills.append(lambda: nc.vector.tensor_add(nv[:], nv[:], ev[:]))
            fills.append(lambda: nc.vector.tensor_add(nv[:], nv[:], empart[:]))
            for bb in range(NSEQ):
                fills.append(
                    lambda bb=bb: nc.vector.tensor_mul(
                        trp[:],
                        aux17_sb[:, cp_base + T * bb : cp_base + T * (bb + 1)],
                        trans_sb,
                    )
                )
                fills.append(
                    lambda bb=bb: nc.vector.reduce_sum(
                        trr[:, bb : bb + 1], trp[:], axis=AX.X
                    )
                )
            for hlf in range(4):
                def aux_lse(hlf=hlf):
                    psE = ps.tile([1, 512], F32, tag="small")
                    nc.tensor.matmul(
                        psE[:, :512], ones_t[:],
                        expEm[:, 2 * hlf : 2 * hlf + 2, :],
                        start=True, stop=True,
                    )
                    nc.scalar.activation(
                        lse[:, 2 * hlf : 2 * hlf + 2, :],
                        psE[:].rearrange("o (b s) -> o b s", b=2),
                        AF.Ln,
                    )
                fills.append(aux_lse)

        emit_fills()

        afin = pp.tile([T, NSEQ], F32, tag="afin")
        lacc = {}
        for ch in range(2):
            for par in range(2):
                lacc[(ch, par)] = pp.tile(
                    [1, 4], F32, tag=f"lacc{ch}{par}", name=f"lacc{ch}{par}"
                )
            nc.vector.memset(lacc[(ch, 0)][:], 0.0)
        ap = ctx.enter_context(tc.tile_pool(name="crf", bufs=4))

        A = {}
        for ch in range(2):
            sl = slice(4 * ch, 4 * ch + 4)
            A[ch] = ap.tile([T, 4], F32, tag=f"A{ch}", name=f"A{ch}")
            nc.vector.tensor_scalar(
                out=A[ch][:], in0=expEm[:, sl, 0], scalar1=expS[:, 0:1],
                scalar2=None, op0=ALU.mult,
            )
        nren_seen = 0
        for t in range(1, S):
            for ch in range(2):
                sl = slice(4 * ch, 4 * ch + 4)
                pt = ps.tile([128, 4], F32, tag="mm")
                nc.tensor.matmul(pt[:T, :], expT[:], A[ch][:], start=True, stop=True)
                A[ch] = ap.tile([T, 4], F32, tag=f"A{ch}", name=f"A{ch}")
                nc.vector.tensor_mul(A[ch][:], pt[:T, :], expEm[:, sl, t])
            if fills:
                fills.pop(0)()
            if t % RENORM == 0:
                for ch in range(2):
                    sl = slice(4 * ch, 4 * ch + 4)
                    psS = ps.tile([1, 512], F32, tag="small")
                    nc.tensor.matmul(
                        psS[:, :4], ones_t[:], A[ch][:], start=True, stop=True
                    )
                    Sr = ap.tile([1, 4], F32, tag=f"Sr{ch}", name=f"Sr{ch}")
                    nc.vector.reciprocal(Sr[:], psS[:, :4])
                    lnS = ap.tile([1, 4], F32, tag=f"lnS{ch}", name=f"lnS{ch}")
                    nc.scalar.activation(lnS[:], psS[:, :4], AF.Ln)
                    pB = ps.tile([128, 4], F32, tag="mm")
                    nc.tensor.matmul(pB[:T, :], ones_1t[:], Sr[:], start=True, stop=True)
                    A2 = ap.tile([T, 4], F32, tag=f"A{ch}", name=f"A{ch}")
                    nc.vector.tensor_mul(A2[:], A[ch][:], pB[:T, :])
                    A[ch] = A2
                    nc.vector.tensor_mul(lnS[:], lnS[:], mren_sb[:, nren_seen, sl])
                    old, new = lacc[(ch, nren_seen % 2)], lacc[(ch, 1 - nren_seen % 2)]
                    nc.vector.tensor_add(new[:], old[:], lnS[:])
                nren_seen += 1
            if t >= S // 2 - 1:
                for ch in range(2):
                    sl = slice(4 * ch, 4 * ch + 4)
                    nc.vector.copy_predicated(
                        afin[:, sl], msel_sb[:, sl, t], A[ch][:]
                    )
        for f in fills:
            f()

        ae = op.tile([T, NSEQ], F32, tag="ae")
        nc.vector.tensor_scalar(
            out=ae[:], in0=afin[:], scalar1=expE[:, 0:1], scalar2=None, op0=ALU.mult
        )
        psZ = ps.tile([1, 512], F32, tag="small")
        nc.tensor.matmul(psZ[:, :NSEQ], ones_t[:], ae[:], start=True, stop=True)
        logZ = sp.tile([1, NSEQ], F32, tag="logZ")
        nc.scalar.activation(logZ[:], psZ[:, :NSEQ], AF.Ln)
        for ch in range(2):
            sl = slice(4 * ch, 4 * ch + 4)
            nc.vector.tensor_add(
                logZ[:, sl], logZ[:, sl], lacc[(ch, nren_seen % 2)][:]
            )

        # numerator wrap-up (trr columns were filled per-seq during the scan)
        nc.vector.tensor_add(nv[:], nv[:], trr[:])
        psN = ps.tile([1, 512], F32, tag="small")
        nc.tensor.matmul(psN[:, :NSEQ], ones_t[:], nv[:], start=True, stop=True)
        nc.vector.tensor_copy(num_sb[:], psN[:, :NSEQ])

        nc.vector.tensor_mul(lse[:], lse[:], vm_sb)
        lsum = sp.tile([1, NSEQ], F32, tag="lsum")
        nc.vector.reduce_sum(lsum[:], lse[:], axis=AX.X)
        psM = ps.tile([1, 512], F32, tag="small")
        nc.tensor.matmul(psM[:, :NSEQ], ones_t[:], empart[:], start=True, stop=True)
        nc.vector.tensor_sub(aux_sb[:], lsum[:], psM[:, :NSEQ])

        dma(out_d[0:1, :], num_sb[:])
        dma(out_d[1:2, :], logZ[:])
        dma(out_d[2:3, :], aux_sb[:])

    nc.compile()
    return nc


PG = np.concatenate(
    [np.arange(2 * H, 3 * H), np.arange(0, H), np.arange(H, 2 * H),
     np.arange(3 * H, 4 * H)]
)  # PyTorch gate rows [i,f,g,o] -> device order [g,i,f,o]


def _prep_maps(inputs):
    bf = ml_dtypes.bfloat16
    f8 = ml_dtypes.float8_e4m3
    x = np.asarray(inputs["x"]).astype(np.int32)
    tags = np.asarray(inputs["tags"]).astype(np.int32)
    emb = np.asarray(inputs["emb"], np.float32)
    sc = np.float32(WSCALE)

    def t_chunks(w, perm=None):
        # (G, K) -> (K//128, 128, G) with gate-row reorder; optional input perm
        w = np.asarray(w, np.float32)[PG]
        K = w.shape[1]
        wT = np.ascontiguousarray(w.T).reshape(K // 128, 128, G)
        if perm is not None:
            wT = wT[perm]
        return wT

    wih0 = [t_chunks(sc * np.asarray(inputs["w_ih_l0"], np.float32)[dd]) for dd in range(2)]
    whh0 = [t_chunks(sc * np.asarray(inputs["w_hh_l0"], np.float32)[dd]) for dd in range(2)]
    whh1 = [t_chunks(sc * np.asarray(inputs["w_hh_l1"], np.float32)[dd]) for dd in range(2)]
    swap = [4, 5, 6, 7, 0, 1, 2, 3]
    wih1 = {}
    for cls in range(2):
        perm = None if cls == 0 else swap
        wih1[cls] = [
            t_chunks(sc * np.asarray(inputs["w_ih_l1"], np.float32)[dd], perm)
            for dd in range(2)
        ]

    PH = np.arange(2 * H)
    PH_swap = np.concatenate([PH[H:], PH[:H]])
    hw_t = np.asarray(inputs["hw_t_w"], np.float32)
    hw_h = np.asarray(inputs["hw_h_w"], np.float32)
    hw_tb = np.asarray(inputs["hw_t_b"], np.float32)
    hw_hb = np.asarray(inputs["hw_h_b"], np.float32)
    fcw = np.asarray(inputs["fc_w"], np.float32)
    hwT, fcwT, hwb = {}, {}, {}
    for cls in range(2):
        pr = PH if cls == 0 else PH_swap
        ht = hw_t[np.ix_(pr, pr)]
        hh = hw_h[np.ix_(pr, pr)]
        hwT[cls] = np.stack(
            [ht.T.reshape(K1, 128, 2 * H), hh.T.reshape(K1, 128, 2 * H)]
        ).astype(bf)
        fcwT[cls] = (
            np.ascontiguousarray(fcw[:, pr].T)
            .reshape(K1, 128, T)
            .transpose(1, 0, 2)
            .reshape(128, K1 * T)
            .astype(bf)
        )
        hwb[cls] = (
            hw_tb[pr].reshape(8, 128).T,
            hw_hb[pr].reshape(8, 128).T,
        )

    b0 = (sc * np.asarray(inputs["b_l0"], np.float32))[:, PG].reshape(2, GC, 128)
    b1 = (sc * np.asarray(inputs["b_l1"], np.float32))[:, PG].reshape(2, GC, 128)
    trans = np.asarray(inputs["crf_trans"], np.float32)
    svec = np.asarray(inputs["crf_start"], np.float32)
    evec = np.asarray(inputs["crf_end"], np.float32)
    fcb = np.asarray(inputs["fc_b"], np.float32)

    valid = tags != 0
    lengths = (x != 0).sum(1)

    maps = []
    for core in range(NC):
        cls = 0 if core < 4 else 1
        dd = cls
        g = core % 4
        sl = slice(g * NSEQ, (g + 1) * NSEQ)
        xl_nat, tl, vl, ll = x[sl], tags[sl], valid[sl], lengths[sl]
        xl = xl_nat if cls == 0 else xl_nat[:, ::-1]
        flat = np.ascontiguousarray(xl).reshape(-1)
        x_idx = np.ascontiguousarray(flat.reshape(TOK // 128, 128).T).astype(np.int32)

        aux128 = np.zeros((128, 48), np.float32)
        aux128[:, 0:16] = b0[dd].reshape(16, 128).T
        aux128[:, 16:32] = b1[dd].reshape(16, 128).T
        aux128[:, 32:40] = hwb[cls][0]
        aux128[:, 40:48] = hwb[cls][1]

        jj = np.arange(T)
        oh = (tl[None, :, :] == jj[:, None, None]) & vl[None, :, :]
        oh_tags = oh.reshape(T, TOK).astype(np.float32)
        cp = np.zeros((T, NSEQ, T), np.float32)
        prev, cur = tl[:, :-1], tl[:, 1:]
        vstep = vl[:, 1:]
        for b in range(NSEQ):
            np.add.at(cp[:, b, :], (prev[b][vstep[b]], cur[b][vstep[b]]), 1.0)
        s0e = np.zeros((T, 2 * NSEQ), np.float32)
        for b in range(NSEQ):
            s0e[tl[b, 0], b] = 1.0
            s0e[tl[b, ll[b] - 1], NSEQ + b] = 1.0
        msel = np.zeros((NSEQ, S), np.float32)
        for b in range(NSEQ):
            msel[b, ll[b] - 1] = 1.0
        msel = np.broadcast_to(msel.reshape(1, TOK), (T, TOK)).astype(np.uint8)
        mren = np.zeros((NREN, NSEQ), np.float32)
        for k in range(NREN):
            mren[k] = (RENORM * (k + 1) <= ll - 1).astype(np.float32)
        aux17 = np.zeros((T, 2220), np.float32)
        aux17[:, 0:T] = trans
        aux17[:, T] = svec
        aux17[:, T + 1] = evec
        aux17[:, T + 2] = fcb
        aux17[:, 20 : 20 + TOK] = oh_tags
        aux17[:, 20 + TOK : 20 + TOK + NSEQ * T] = cp.reshape(T, NSEQ * T)
        aux17[:, 20 + TOK + NSEQ * T : 20 + TOK + NSEQ * T + 2 * NSEQ] = s0e
        vm = vl.reshape(1, TOK).astype(np.uint8)
        prow = np.arange(128, dtype=np.int32).reshape(128, 1)
        if cls == 0:
            prow = prow + 128

        maps.append(
            dict(
                x_idx=x_idx,
                emb=emb,
                wih0T=wih0[dd].astype(bf),
                whh0T=whh0[dd].astype(f8 if USE_FP8 else bf),
                wih1T=wih1[cls][dd].astype(bf),
                whh1T=whh1[dd].astype(f8 if USE_FP8 else bf),
                hwT=hwT[cls],
                fcwT=fcwT[cls],
                aux128=aux128,
                aux17=aux17,
                mren=mren.reshape(1, -1),
                vm=vm,
                msel=msel,
                prow=prow,
            )
        )
    return maps, valid


TRACE = {}


def kernel(**inputs):
    if "nc" not in _CACHE:
        _CACHE["nc"] = _build_nc()
    nc = _CACHE["nc"]
    maps, valid = _prep_maps(inputs)
    kw = {}
    if TRACE.get("on"):
        kw = dict(trace=True, tmpdir=TRACE.get("dir"), trace_cores=[0])
    res = run_bass_kernel_spmd(nc, maps, list(range(NC)), **kw)
    TRACE["last"] = res
    outs = [res.results[i]["out"] for i in range(4)]
    num = np.concatenate([o[0] for o in outs])
    logZ = np.concatenate([o[1] for o in outs])
    aux = np.concatenate([o[2] for o in outs])
    crf_loss = -np.mean(num - logZ, dtype=np.float32)
    aux_loss = np.float32(aux.sum()) / np.float32(max(valid.sum(), 1))
    return np.float32(crf_loss + np.float32(0.1) * aux_loss)
